# revision 1
# baseline (speedup 1.0000x reference)
"""GAT (3-layer) + edge MLP on 8 TRN2 NeuronCores.

Strategy: dst-sorted edge sharding into per-core node slices; greedy
<=128-node blocks with fixed TLO+THI (lo/hi table half) 128-edge tiles;
segment softmax-sum via indicator-matrix matmuls accumulated in PSUM; bf16
tables gathered with int16 dma_gather; three small AllGathers between layers.
"""
import sys
sys.path.insert(0, '/opt/trn_rl_repo')
import numpy as np
import ml_dtypes

BF = ml_dtypes.bfloat16

F_IN = 128
NEG = 0.2

N = 50000
E = 400000
NCORES = 8
SL = 6256
NP = SL * NCORES
HALF = NP // 2
TLO = 5
THI = 5
TT = TLO + THI
G = 4                    # blocks per gather group
GF = 4                   # final-stage tiles per gather group


def set_small():
    global N, E, SL, NP, HALF, TLO, THI, TT, G, GF
    N, E = 4096, 32768
    SL = 512
    NP = SL * NCORES
    HALF = NP // 2
    TLO = THI = 2
    TT = TLO + THI
    G = 4
    GF = 4


def _wrap16(idx, pad=0):
    idx = np.asarray(idx, np.int64)
    n = len(idx)
    cols = (n + 15) // 16
    a = np.full((16, cols), pad, np.int16)
    a[np.arange(n) % 16, np.arange(n) // 16] = idx
    return np.tile(a, (8, 1))


def _bcast_rows(v, parts=128):
    v = np.asarray(v, np.float32).reshape(-1)
    return np.broadcast_to(v, (parts, v.size)).copy()


def _host_prep(x, edge_index, edge_attr, year, quarter,
               W1, a1s, a1d, b1, W2, a2s, a2d, b2, W3, a3s, a3d, b3,
               fc1_w, fc1_b, fc2_w, fc2_b):
    n = x.shape[0]
    loops = np.arange(n, dtype=np.int64)
    src = np.concatenate([edge_index[0].astype(np.int64), loops])
    dst = np.concatenate([edge_index[1].astype(np.int64), loops])
    order = np.argsort(dst, kind='stable')
    src_s, dst_s = src[order], dst[order]

    counts = np.bincount(dst_s, minlength=NP)
    starts = np.zeros(NP + 1, np.int64)
    np.cumsum(counts, out=starts[1:])

    per_core_blocks = []
    for k in range(NCORES):
        blocks = []
        node = k * SL
        end = min((k + 1) * SL, n)
        while node < end:
            bs = node
            lo_e, hi_e, dl_lo, dl_hi = [], [], [], []
            cnt = 0
            while node < end and cnt < 128:
                e0, e1 = starts[node], starts[node + 1]
                seg = src_s[e0:e1]
                seg_lo = seg[seg < HALF]
                seg_hi = seg[seg >= HALF]
                if len(lo_e) + len(seg_lo) > TLO * 128 or \
                   len(hi_e) + len(seg_hi) > THI * 128:
                    break
                lo_e.extend(seg_lo.tolist())
                dl_lo.extend([node - bs] * len(seg_lo))
                hi_e.extend((seg_hi - HALF).tolist())
                dl_hi.extend([node - bs] * len(seg_hi))
                node += 1
                cnt += 1
            assert cnt > 0
            blocks.append((bs, cnt, lo_e, hi_e, dl_lo, dl_hi))
        per_core_blocks.append(blocks)

    Bmax = max(len(b) for b in per_core_blocks)
    B = ((Bmax + G - 1) // G) * G
    NG = B // G

    EC = E // NCORES
    TF = (EC + 127) // 128
    TFP = ((TF + GF - 1) // GF) * GF
    NGF = TFP // GF
    OUTN = ((TFP * 128 + 16383) // 16384) * 16384

    H1, C1 = a1s.shape
    H2, C2 = a2s.shape
    W1r = W1.reshape(H1, C1, F_IN)
    w1t_packed = np.stack([W1r[h].T for h in range(H1)], 0)      # [4,128,128]
    w1s = np.einsum('hc,hcf->fh', a1s, W1r)
    w1d = np.einsum('hc,hcf->fh', a1d, W1r)
    shared = dict(
        w1t=np.concatenate([w1t_packed[h] for h in range(H1)], 1).astype(BF),
        w1s_bc=_bcast_rows(w1s.T.reshape(-1)).astype(BF),
        w1d_bc=_bcast_rows(w1d.T.reshape(-1)).astype(BF),
        w2t=W2.T.reshape(4, 128, H2 * C2).transpose(1, 0, 2).reshape(128, 512).astype(BF),
        a2s_bc=_bcast_rows(a2s.reshape(-1)).astype(BF),
        a2d_bc=_bcast_rows(a2d.reshape(-1)).astype(BF),
        w3t=W3.T.astype(BF),
        w3s_bc=_bcast_rows(a3s.reshape(-1)).astype(BF),
        w3d_bc=_bcast_rows(a3d.reshape(-1)).astype(BF),
        ab=np.concatenate([fc1_w[:, 0:8].T, fc1_w[:, 8:16].T], 1).astype(np.float32),
        cb=_bcast_rows(np.concatenate([fc1_w[:, 16], fc1_w[:, 17],
                                       fc1_w[:, 18], fc1_b])),
        w2b=_bcast_rows(fc2_w.reshape(-1)),
        b1_bc=_bcast_rows(b1), b2_bc=_bcast_rows(b2), b3_bc=_bcast_rows(b3),
        iota=np.broadcast_to(np.arange(128, dtype=np.float32), (128, 128)).copy(),
        ident_bf=np.eye(128, dtype=BF),
        ident_f=np.eye(128, dtype=np.float32),
    )
    x_pad = np.zeros((NP, F_IN), np.float32)
    x_pad[:n] = x
    x_bf = x_pad.astype(BF)

    in_maps = []
    for k in range(NCORES):
        blocks = per_core_blocks[k]
        lo_idx = np.zeros((NG, 128, G * TLO * 8), np.int16)
        hi_idx = np.zeros((NG, 128, G * THI * 8), np.int16)
        dstloc = np.full((128, B * TT), -1.0, np.float32)
        rows_all = np.full((B, 128), SL, np.int64)
        for b in range(B):
            if b < len(blocks):
                bs, cnt, lo_e, hi_e, dl_lo, dl_hi = blocks[b]
                for i, d in enumerate(dl_lo):
                    dstloc[i % 128, b * TT + i // 128] = d
                for i, d in enumerate(dl_hi):
                    j = TLO * 128 + i
                    dstloc[j % 128, b * TT + j // 128] = d
                rows_all[b, :cnt] = np.arange(bs, bs + cnt) - k * SL
        for g in range(NG):
            lo_cat, hi_cat, row_cat = [], [], []
            for b in range(g * G, (g + 1) * G):
                le = blocks[b][2] if b < len(blocks) else []
                he = blocks[b][3] if b < len(blocks) else []
                lo_cat.extend(le + [0] * (TLO * 128 - len(le)))
                hi_cat.extend(he + [0] * (THI * 128 - len(he)))
                row_cat.extend(rows_all[b].tolist())
            lo_idx[g] = _wrap16(lo_cat)
            hi_idx[g] = _wrap16(hi_cat)
            if g == 0:
                blkrow_idx = np.zeros((NG, 128, G * 8), np.int16)
            blkrow_idx[g] = _wrap16(row_cat)

        fe0 = k * EC
        fsrc_p = np.zeros(TFP * 128, np.int64)
        fdst_p = np.zeros(TFP * 128, np.int64)
        fsrc_p[:EC] = src[fe0:fe0 + EC]
        fdst_p[:EC] = dst[fe0:fe0 + EC]
        fsrc_q = np.zeros((NGF, 128, GF * 8), np.int16)
        fdst_q = np.zeros((NGF, 128, GF * 8), np.int16)
        for g in range(NGF):
            s0 = g * GF * 128
            fsrc_q[g] = _wrap16(fsrc_p[s0:s0 + GF * 128] >> 1)
            fdst_q[g] = _wrap16(fdst_p[s0:s0 + GF * 128] >> 1)
        msrc = (fsrc_p & 1).astype(np.int8).reshape(TFP, 128).T.copy()
        mdst = (fdst_p & 1).astype(np.int8).reshape(TFP, 128).T.copy()
        att = np.zeros((TFP * 128, 4), np.float32)
        att[:EC, 0] = edge_attr.reshape(-1)[fe0:fe0 + EC]
        att[:EC, 1] = year.reshape(-1)[fe0:fe0 + EC]
        att[:EC, 2] = quarter.reshape(-1)[fe0:fe0 + EC]
        att[:EC, 3] = 1.0
        att = att.reshape(TFP, 128, 4).transpose(1, 0, 2).reshape(128, TFP * 4).copy()

        m = dict(shared)
        m.update(
            x_slice=np.vstack([x_bf[k * SL:(k + 1) * SL],
                               np.zeros((1, F_IN), BF)]),
            lo_idx=lo_idx, hi_idx=hi_idx, dstloc=dstloc, blkrow=blkrow_idx,
            fsrc=fsrc_q, fdst=fdst_q, msrc=msrc, mdst=mdst, attr=att,
        )
        in_maps.append(m)

    meta = dict(B=B, NG=NG, TFP=TFP, NGF=NGF, OUTN=OUTN, EC=EC,
                fc2_b=float(np.asarray(fc2_b).reshape(())))
    return in_maps, meta


def _build(meta):
    from concourse import bass, bacc, mybir
    import concourse.tile as tile
    F32 = mybir.dt.float32
    BF16 = mybir.dt.bfloat16
    I16 = mybir.dt.int16
    AF = mybir.ActivationFunctionType
    OP = mybir.AluOpType
    AX = mybir.AxisListType

    B, NG, TFP, NGF, OUTN = (meta['B'], meta['NG'], meta['TFP'],
                             meta['NGF'], meta['OUTN'])
    FC2B = meta['fc2_b']

    nc = bacc.Bacc(None, num_devices=NCORES, target_bir_lowering=False)
    P = lambda name, shape, dt: nc.declare_dram_parameter(name, shape, dt, isOutput=False)

    x_slice = P("x_slice", [SL + 1, 128], BF16)
    w1t = P("w1t", [128, 512], BF16)
    w1s_bc = P("w1s_bc", [128, 512], BF16)
    w1d_bc = P("w1d_bc", [128, 512], BF16)
    w2t = P("w2t", [128, 512], BF16)
    a2s_bc = P("a2s_bc", [128, 128], BF16)
    a2d_bc = P("a2d_bc", [128, 128], BF16)
    w3t = P("w3t", [128, 8], BF16)
    w3s_bc = P("w3s_bc", [128, 8], BF16)
    w3d_bc = P("w3d_bc", [128, 8], BF16)
    ab = P("ab", [8, 32], F32)
    cb = P("cb", [128, 64], F32)
    w2b = P("w2b", [128, 16], F32)
    b1_bc = P("b1_bc", [128, 512], F32)
    b2_bc = P("b2_bc", [128, 128], F32)
    b3_bc = P("b3_bc", [128, 8], F32)
    iota_in = P("iota", [128, 128], F32)
    ident_bf_in = P("ident_bf", [128, 128], BF16)
    ident_f_in = P("ident_f", [128, 128], F32)
    lo_idx = P("lo_idx", [NG, 128, G * TLO * 8], I16)
    hi_idx = P("hi_idx", [NG, 128, G * THI * 8], I16)
    dstloc = P("dstloc", [128, B * TT], F32)
    blkrow = P("blkrow", [NG, 128, G * 8], I16)
    fsrc = P("fsrc", [NGF, 128, GF * 8], I16)
    fdst = P("fdst", [NGF, 128, GF * 8], I16)
    msrc = P("msrc", [128, TFP], mybir.dt.int8)
    mdst = P("mdst", [128, TFP], mybir.dt.int8)
    attr = P("attr", [128, TFP * 4], F32)

    out_final = nc.declare_dram_parameter("out_final", [OUTN], F32, isOutput=True)

    x_bounce = nc.dram_tensor("x_bounce", [SL, 128], BF16)
    h2_slice = nc.dram_tensor("h2_slice", [SL + 1, 128], BF16)
    h3_slice = nc.dram_tensor("h3_slice", [SL + 1, 128], BF16)
    h3o_slice = nc.dram_tensor("h3o_slice", [SL + 1, 64], F32)
    h3_comp = nc.dram_tensor("h3_comp", [SL, 8], BF16)
    h3o_comp = nc.dram_tensor("h3o_comp", [SL, 8], F32)
    t_x = nc.dram_tensor("t_x", [NP, 128], BF16, addr_space="Shared")
    t_h2 = nc.dram_tensor("t_h2", [NP, 128], BF16, addr_space="Shared")
    h3_ag = nc.dram_tensor("h3_ag", [NP, 8], BF16, addr_space="Shared")
    h3o_ag = nc.dram_tensor("h3o_ag", [NP, 8], F32, addr_space="Shared")
    t_h3 = nc.dram_tensor("t_h3", [NP, 128], BF16)
    t_u = nc.dram_tensor("t_u", [NP // 2, 64], F32)
    t_v = nc.dram_tensor("t_v", [NP // 2, 64], F32)

    with tile.TileContext(nc) as tc:
        with tc.tile_pool(name="const", bufs=1) as cp, \
             tc.tile_pool(name="stage", bufs=2) as sp, \
             tc.tile_pool(name="work", bufs=2) as wp, \
             tc.tile_pool(name="psA", bufs=2, space="PSUM") as psA, \
             tc.tile_pool(name="psB", bufs=2, space="PSUM") as psB, \
             tc.tile_pool(name="psAcc", bufs=1, space="PSUM") as psAcc:

            _cn = [0]
            def load_const(ap, shape, dt):
                _cn[0] += 1
                t = cp.tile(shape, dt, tag=f"const{_cn[0]}")
                nc.sync.dma_start(out=t[:], in_=ap)
                return t

            c_w1t = load_const(w1t[:, :], [128, 512], BF16)
            c_w1s = load_const(w1s_bc[:, :], [128, 512], BF16)
            c_w1d = load_const(w1d_bc[:, :], [128, 512], BF16)
            c_w2t = load_const(w2t[:, :], [128, 512], BF16)
            c_a2s = load_const(a2s_bc[:, :], [128, 128], BF16)
            c_a2d = load_const(a2d_bc[:, :], [128, 128], BF16)
            c_w3t = load_const(w3t[:, :], [128, 8], BF16)
            c_w3s = load_const(w3s_bc[:, :], [128, 8], BF16)
            c_w3d = load_const(w3d_bc[:, :], [128, 8], BF16)
            c_ab = load_const(ab[:, :], [8, 32], F32)
            c_cb = load_const(cb[:, :], [128, 64], F32)
            c_w2b = load_const(w2b[:, :], [128, 16], F32)
            c_b1 = load_const(b1_bc[:, :], [128, 512], F32)
            c_b2 = load_const(b2_bc[:, :], [128, 128], F32)
            c_b3 = load_const(b3_bc[:, :], [128, 8], F32)
            c_iota = load_const(iota_in[:, :], [128, 128], F32)
            c_idbf = load_const(ident_bf_in[:, :], [128, 128], BF16)
            c_idf = load_const(ident_f_in[:, :], [128, 128], F32)

            # zero scatter-target slices
            zt = cp.tile([128, 128], F32)
            nc.vector.memset(zt[:], 0.0)
            for tbl, wcols in ((h2_slice, 64), (h3_slice, 64), (h3o_slice, 64)):
                view = tbl[:, :].bitcast(F32)
                rows = SL + 1
                for r0 in range(0, rows, 128):
                    r1 = min(r0 + 128, rows)
                    nc.sync.dma_start(out=view[r0:r1, :], in_=zt[:r1 - r0, :wcols])

            def elu(o_ap, bias_ap, width):
                m = wp.tile([128, width], F32, tag=f"elm{width}")
                nc.vector.tensor_add(out=m[:], in0=o_ap, in1=bias_ap)
                ei = wp.tile([128, width], F32, tag=f"eli{width}")
                nc.vector.tensor_scalar(out=ei[:], in0=m[:], scalar1=0.0,
                                        scalar2=None, op0=OP.min)
                e = wp.tile([128, width], F32, tag=f"ele{width}")
                nc.scalar.activation(e[:], ei[:], AF.Exp)
                r = wp.tile([128, width], F32, tag=f"elr{width}")
                nc.vector.tensor_scalar(out=r[:], in0=m[:], scalar1=0.0,
                                        scalar2=-1.0, op0=OP.max, op1=OP.add)
                out = wp.tile([128, width], F32, tag=f"elo{width}")
                nc.vector.tensor_add(out=out[:], in0=e[:], in1=r[:])
                return out

            def edge_layer(layer, src_tbl, blk_tbl, scat_tbl, scat_elem):
                """Emit one GAT layer. Returns nothing; writes scat_tbl."""
                nH = 4 if layer != 3 else 1
                for g in range(NG):
                    li = sp.tile([128, G * TLO * 8], I16, tag="li")
                    nc.sync.dma_start(out=li[:], in_=lo_idx[g])
                    hi = sp.tile([128, G * THI * 8], I16, tag="hi")
                    nc.sync.dma_start(out=hi[:], in_=hi_idx[g])
                    bi = sp.tile([128, G * 8], I16, tag="bi")
                    nc.sync.dma_start(out=bi[:], in_=blkrow[g])
                    dl = sp.tile([128, G * TT], F32, tag="dl")
                    nc.sync.dma_start(out=dl[:], in_=dstloc[:, g * G * TT:(g + 1) * G * TT])
                    stag_lo = sp.tile([128, G * TLO * 128], BF16, tag="stag_lo")
                    stag_hi = sp.tile([128, G * THI * 128], BF16, tag="stag_hi")
                    for bg in range(G):
                        nc.gpsimd.dma_gather(
                            out_ap=stag_lo[:, bg * TLO * 128:(bg + 1) * TLO * 128]
                                .rearrange("p (a b) -> p a b", a=TLO),
                            in_ap=src_tbl[0:HALF, :],
                            idxs_ap=li[:, bg * TLO * 8:(bg + 1) * TLO * 8],
                            num_idxs=TLO * 128, num_idxs_reg=TLO * 128,
                            elem_size=128)
                        nc.gpsimd.dma_gather(
                            out_ap=stag_hi[:, bg * THI * 128:(bg + 1) * THI * 128]
                                .rearrange("p (a b) -> p a b", a=THI),
                            in_ap=src_tbl[HALF:NP, :],
                            idxs_ap=hi[:, bg * THI * 8:(bg + 1) * THI * 8],
                            num_idxs=THI * 128, num_idxs_reg=THI * 128,
                            elem_size=128)
                    brow = sp.tile([128, G * 128], BF16, tag="brow")
                    nc.gpsimd.dma_gather(
                        out_ap=brow[:].rearrange("p (a b) -> p a b", a=G),
                        in_ap=blk_tbl[:, :], idxs_ap=bi[:],
                        num_idxs=G * 128, num_idxs_reg=G * 128, elem_size=128)

                    owid = 128 if layer != 3 else 64
                    ostage = sp.tile([128, G * owid],
                                     BF16 if layer != 3 else F32, tag="ost")

                    for bg in range(G):
                        xb = brow[:, bg * 128:(bg + 1) * 128]
                        # --- d per block node ---
                        dblk = wp.tile([128, nH], F32, tag="dblk")
                        if layer == 1:
                            tmp = wp.tile([128, 512], F32, tag="dtmp")
                            nc.vector.tensor_tensor(
                                out=tmp[:],
                                in0=xb.rearrange("p (o f) -> p o f", o=1).to_broadcast([128, 4, 128]),
                                in1=c_w1d[:].rearrange("p (h f) -> p h f", h=4),
                                op=OP.mult)
                            nc.vector.tensor_reduce(
                                out=dblk[:], in_=tmp[:].rearrange("p (h f) -> p h f", h=4),
                                axis=AX.X, op=OP.add)
                        elif layer == 2:
                            tmp = wp.tile([128, 128], F32, tag="dtmp2")
                            nc.vector.tensor_tensor(out=tmp[:], in0=xb, in1=c_a2d[:], op=OP.mult)
                            nc.vector.tensor_reduce(
                                out=dblk[:], in_=tmp[:].rearrange("p (h f) -> p h f", h=4),
                                axis=AX.X, op=OP.add)
                        else:
                            tmp = wp.tile([128, 8], F32, tag="dtmp3")
                            nc.vector.tensor_tensor(out=tmp[:], in0=xb[:, 0:8], in1=c_w3d[:], op=OP.mult)
                            nc.vector.tensor_reduce(out=dblk[:], in_=tmp[:], axis=AX.X, op=OP.add)
                        dblk_bf = wp.tile([128, nH], BF16, tag="dblk_bf")
                        nc.vector.tensor_copy(out=dblk_bf[:], in_=dblk[:])

                        # --- pass 0 over tiles: m0 cache, ex cache, den ---
                        m0c = wp.tile([128, TT * 128], BF16, tag="m0c")
                        exc = wp.tile([128, TT * nH], F32, tag="exc")
                        den_ps = psAcc.tile([128, nH], F32, tag="den")
                        if layer != 1:
                            num_w = 132 if layer == 2 else 9
                            acc_ps = psAcc.tile([128, num_w], F32, tag="acc")
                        for t in range(TT):
                            sl0 = (bg * TLO + t) * 128 if t < TLO else (bg * THI + (t - TLO)) * 128
                            xg = (stag_lo if t < TLO else stag_hi)[:, sl0:sl0 + 128]
                            dcol = dl[:, bg * TT + t:bg * TT + t + 1]
                            m0 = m0c[:, t * 128:(t + 1) * 128]
                            nc.vector.tensor_tensor(
                                out=m0, in0=dcol.to_broadcast([128, 128]),
                                in1=c_iota[:], op=OP.is_equal)
                            m0t_ps = psB.tile([128, 128], BF16, tag="b")
                            nc.tensor.transpose(out=m0t_ps[:], in_=m0, identity=c_idbf[:])
                            m0t = wp.tile([128, 128], BF16, tag="m0t_sb")
                            nc.vector.tensor_copy(out=m0t[:], in_=m0t_ps[:])
                            de_ps = psA.tile([128, nH], F32, tag="a")
                            nc.tensor.matmul(de_ps[:], m0t[:], dblk_bf[:], start=True, stop=True)
                            sg = wp.tile([128, nH], F32, tag="sg")
                            if layer == 1:
                                tmp2 = wp.tile([128, 512], F32, tag="stmp")
                                nc.vector.tensor_tensor(
                                    out=tmp2[:],
                                    in0=xg.rearrange("p (o f) -> p o f", o=1).to_broadcast([128, 4, 128]),
                                    in1=c_w1s[:].rearrange("p (h f) -> p h f", h=4), op=OP.mult)
                                nc.vector.tensor_reduce(
                                    out=sg[:], in_=tmp2[:].rearrange("p (h f) -> p h f", h=4),
                                    axis=AX.X, op=OP.add)
                            elif layer == 2:
                                tmp2 = wp.tile([128, 128], F32, tag="stmp2")
                                nc.vector.tensor_tensor(out=tmp2[:], in0=xg, in1=c_a2s[:], op=OP.mult)
                                nc.vector.tensor_reduce(
                                    out=sg[:], in_=tmp2[:].rearrange("p (h f) -> p h f", h=4),
                                    axis=AX.X, op=OP.add)
                            else:
                                tmp2 = wp.tile([128, 8], F32, tag="stmp3")
                                nc.vector.tensor_tensor(out=tmp2[:], in0=xg[:, 0:8], in1=c_w3s[:], op=OP.mult)
                                nc.vector.tensor_reduce(out=sg[:], in_=tmp2[:], axis=AX.X, op=OP.add)
                            raw = wp.tile([128, nH], F32, tag="raw")
                            nc.vector.tensor_add(out=raw[:], in0=sg[:], in1=de_ps[:])
                            lr = wp.tile([128, nH], F32, tag="lr")
                            nc.vector.scalar_tensor_tensor(
                                out=lr[:], in0=raw[:], scalar=NEG, in1=raw[:],
                                op0=OP.mult, op1=OP.max)
                            ex = wp.tile([128, nH], F32, tag="ex")
                            nc.scalar.activation(ex[:], lr[:], AF.Exp)
                            exd = exc[:, t * nH:(t + 1) * nH]
                            nc.vector.tensor_copy(out=exd, in_=ex[:])
                            ex_bf = wp.tile([128, nH], BF16, tag="ex_bf")
                            nc.vector.tensor_copy(out=ex_bf[:], in_=ex[:])
                            first, last = (t == 0), (t == TT - 1)
                            nc.tensor.matmul(den_ps[:], m0, ex_bf[:], start=first, stop=last)
                            if layer != 1:
                                gw = wp.tile([128, num_w], BF16, tag="gw")
                                if layer == 2:
                                    nc.vector.tensor_tensor(
                                        out=gw[:, 0:128].rearrange("p (h c) -> p h c", h=4),
                                        in0=xg.rearrange("p (h c) -> p h c", h=4),
                                        in1=ex_bf[:].rearrange("p (h o) -> p h o", o=1).to_broadcast([128, 4, 32]),
                                        op=OP.mult)
                                    nc.vector.tensor_copy(out=gw[:, 128:132], in_=ex_bf[:])
                                else:
                                    nc.vector.tensor_scalar(
                                        out=gw[:, 0:8], in0=xg[:, 0:8],
                                        scalar1=exd[:, 0:1], scalar2=None, op0=OP.mult)
                                    nc.vector.tensor_copy(out=gw[:, 8:9], in_=ex_bf[:])
                                nc.tensor.matmul(acc_ps[:], m0, gw[:], start=first, stop=last)

                        den_sb = wp.tile([128, nH], F32, tag="den_sb")
                        nc.vector.tensor_scalar(out=den_sb[:], in0=den_ps[:], scalar1=1e-30,
                                                 scalar2=None, op0=OP.max)
                        rec = wp.tile([128, nH], F32, tag="rec")
                        nc.vector.reciprocal(out=rec[:], in_=den_sb[:])

                        if layer == 1:
                            o_sb = wp.tile([128, 512], F32, tag="o_sb")
                            for h in range(4):
                                p_ps = psAcc.tile([128, 128], F32, tag="p1")
                                for t in range(TT):
                                    sl0 = (bg * TLO + t) * 128 if t < TLO else (bg * THI + (t - TLO)) * 128
                                    xg = (stag_lo if t < TLO else stag_hi)[:, sl0:sl0 + 128]
                                    mw = wp.tile([128, 128], BF16, tag="mw")
                                    nc.vector.tensor_scalar(
                                        out=mw[:], in0=m0c[:, t * 128:(t + 1) * 128],
                                        scalar1=exc[:, t * nH + h:t * nH + h + 1],
                                        scalar2=None, op0=OP.mult)
                                    nc.tensor.matmul(p_ps[:], mw[:], xg,
                                                     start=(t == 0), stop=(t == TT - 1))
                                pc = wp.tile([128, 128], BF16, tag="pc")
                                nc.vector.tensor_copy(out=pc[:], in_=p_ps[:])
                                pt_ps = psB.tile([128, 128], BF16, tag="b")
                                nc.tensor.transpose(out=pt_ps[:], in_=pc[:], identity=c_idbf[:])
                                pt = wp.tile([128, 128], BF16, tag="pt_sb")
                                nc.vector.tensor_copy(out=pt[:], in_=pt_ps[:])
                                nh_ps = psA.tile([128, 128], F32, tag="a")
                                nc.tensor.matmul(nh_ps[:], pt[:], c_w1t[:, h * 128:(h + 1) * 128],
                                                 start=True, stop=True)
                                nc.vector.tensor_scalar(
                                    out=o_sb[:, h * 128:(h + 1) * 128], in0=nh_ps[:],
                                    scalar1=rec[:, h:h + 1], scalar2=None, op0=OP.mult)
                            elu1 = elu(o_sb[:], c_b1[:], 512)
                            # dense2 -> h2 block
                            h2_ps = psA.tile([128, 128], F32, tag="a")
                            for c in range(4):
                                cc = wp.tile([128, 128], BF16, tag="cc")
                                nc.vector.tensor_copy(out=cc[:], in_=elu1[:, c * 128:(c + 1) * 128])
                                ct_ps = psB.tile([128, 128], BF16, tag="b")
                                nc.tensor.transpose(out=ct_ps[:], in_=cc[:], identity=c_idbf[:])
                                ct = wp.tile([128, 128], BF16, tag="ct_sb")
                                nc.vector.tensor_copy(out=ct[:], in_=ct_ps[:])
                                nc.tensor.matmul(h2_ps[:], ct[:], c_w2t[:, c * 128:(c + 1) * 128],
                                                 start=(c == 0), stop=(c == 3))
                            nc.vector.tensor_copy(out=ostage[:, bg * 128:(bg + 1) * 128],
                                                  in_=h2_ps[:])
                        elif layer == 2:
                            o_sb = wp.tile([128, 128], F32, tag="o_sb2")
                            for h in range(4):
                                nc.vector.tensor_scalar(
                                    out=o_sb[:, h * 32:(h + 1) * 32],
                                    in0=acc_ps[:, h * 32:(h + 1) * 32],
                                    scalar1=rec[:, h:h + 1], scalar2=None, op0=OP.mult)
                            elu2 = elu(o_sb[:], c_b2[:], 128)
                            cc = wp.tile([128, 128], BF16, tag="cc")
                            nc.vector.tensor_copy(out=cc[:], in_=elu2[:])
                            ct_ps = psB.tile([128, 128], BF16, tag="b")
                            nc.tensor.transpose(out=ct_ps[:], in_=cc[:], identity=c_idbf[:])
                            ct = wp.tile([128, 128], BF16, tag="ct_sb")
                            nc.vector.tensor_copy(out=ct[:], in_=ct_ps[:])
                            h3_ps = psA.tile([128, 8], F32, tag="a")
                            nc.tensor.matmul(h3_ps[:], ct[:], c_w3t[:], start=True, stop=True)
                            st = ostage[:, bg * 128:(bg + 1) * 128]
                            nc.vector.memset(st, 0.0)
                            nc.vector.tensor_copy(out=ostage[:, bg * 128:bg * 128 + 8],
                                                  in_=h3_ps[:])
                        else:
                            o_sb = wp.tile([128, 8], F32, tag="o_sb3")
                            nc.vector.tensor_scalar(
                                out=o_sb[:], in0=acc_ps[:, 0:8],
                                scalar1=rec[:, 0:1], scalar2=None, op0=OP.mult)
                            elu3 = elu(o_sb[:], c_b3[:], 8)
                            st = ostage[:, bg * 64:(bg + 1) * 64]
                            nc.vector.memset(st, 0.0)
                            nc.vector.tensor_copy(out=ostage[:, bg * 64:bg * 64 + 8],
                                                  in_=elu3[:])

                    nc.gpsimd.dma_scatter_add(
                        scat_tbl[:, :], ostage[:].rearrange("p (a b) -> p a b", a=G),
                        bi[:], G * 128, G * 128, scat_elem)

            # ======== layers ========
            nc.sync.dma_start(out=x_bounce[:, :], in_=x_slice[0:SL, :])
            nc.gpsimd.collective_compute(
                "AllGather", mybir.AluOpType.bypass,
                replica_groups=[list(range(NCORES))],
                ins=[x_bounce[:, :]], outs=[t_x[:, :]])
            edge_layer(1, t_x, x_slice, h2_slice, 128)
            nc.gpsimd.collective_compute(
                "AllGather", mybir.AluOpType.bypass,
                replica_groups=[list(range(NCORES))],
                ins=[h2_slice[0:SL, :]], outs=[t_h2[:, :]])

            edge_layer(2, t_h2, h2_slice, h3_slice, 128)
            for r0 in range(0, SL, 512):
                r1 = min(r0 + 512, SL)
                nc.sync.dma_start(out=h3_comp[r0:r1, :],
                                  in_=h3_slice[r0:r1, 0:8])
            nc.gpsimd.collective_compute(
                "AllGather", mybir.AluOpType.bypass,
                replica_groups=[list(range(NCORES))],
                ins=[h3_comp[:, :]], outs=[h3_ag[:, :]])
            for r0 in range(0, NP, 512):
                r1 = min(r0 + 512, NP)
                nc.sync.dma_start(out=t_h3[r0:r1, 0:8],
                                  in_=h3_ag[r0:r1, :])

            edge_layer(3, t_h3, h3_slice, h3o_slice, 64)
            for r0 in range(0, SL, 512):
                r1 = min(r0 + 512, SL)
                nc.sync.dma_start(out=h3o_comp[r0:r1, :],
                                  in_=h3o_slice[r0:r1, 0:8])
            nc.gpsimd.collective_compute(
                "AllGather", mybir.AluOpType.bypass,
                replica_groups=[list(range(NCORES))],
                ins=[h3o_comp[:, :]], outs=[h3o_ag[:, :]])

            # ======== u/v tables (pack-2 rows) ========
            for r in range(NP // 128):
                hrows = wp.tile([128, 8], F32, tag="hrows")
                nc.sync.dma_start(out=hrows[:], in_=h3o_ag[r * 128:(r + 1) * 128, :])
                ht_ps = psA.tile([128, 128], F32, tag="a")
                nc.tensor.transpose(out=ht_ps[0:8, :], in_=hrows[:], identity=c_idf[:])
                ht = wp.tile([8, 128], F32, tag="ht_sb")
                nc.vector.tensor_copy(out=ht[:], in_=ht_ps[0:8, :])
                uv_ps = psA.tile([128, 32], F32, tag="a")
                nc.tensor.matmul(uv_ps[:], ht[:], c_ab[:], start=True, stop=True)
                ust = wp.tile([128, 16], F32, tag="ust")
                nc.vector.tensor_copy(out=ust[:], in_=uv_ps[:, 0:16])
                vst = wp.tile([128, 16], F32, tag="vst")
                nc.vector.tensor_copy(out=vst[:], in_=uv_ps[:, 16:32])
                du = t_u[:, :].rearrange("r (s c) -> r s c", s=2)
                nc.sync.dma_start(out=du[r * 64:(r + 1) * 64, :, 0:16], in_=ust[:])
                dv = t_v[:, :].rearrange("r (s c) -> r s c", s=2)
                nc.sync.dma_start(out=dv[r * 64:(r + 1) * 64, :, 0:16], in_=vst[:])

            # ======== final edge MLP ========
            fout = sp.tile([128, 128], F32, tag="fout")
            nsup = 0
            for g in range(NGF):
                fi = sp.tile([128, GF * 8], I16, tag="fi")
                nc.sync.dma_start(out=fi[:], in_=fsrc[g])
                di = sp.tile([128, GF * 8], I16, tag="di")
                nc.sync.dma_start(out=di[:], in_=fdst[g])
                ug = sp.tile([128, GF * 64], F32, tag="ug")
                nc.gpsimd.dma_gather(
                    out_ap=ug[:].rearrange("p (a b) -> p a b", a=GF),
                    in_ap=t_u[:, :], idxs_ap=fi[:],
                    num_idxs=GF * 128, num_idxs_reg=GF * 128, elem_size=64)
                vg = sp.tile([128, GF * 64], F32, tag="vg")
                nc.gpsimd.dma_gather(
                    out_ap=vg[:].rearrange("p (a b) -> p a b", a=GF),
                    in_ap=t_v[:, :], idxs_ap=di[:],
                    num_idxs=GF * 128, num_idxs_reg=GF * 128, elem_size=64)
                ms = sp.tile([128, GF], mybir.dt.int8, tag="ms")
                nc.sync.dma_start(out=ms[:], in_=msrc[:, g * GF:(g + 1) * GF])
                md = sp.tile([128, GF], mybir.dt.int8, tag="md")
                nc.sync.dma_start(out=md[:], in_=mdst[:, g * GF:(g + 1) * GF])
                at = sp.tile([128, GF * 4], F32, tag="at")
                nc.sync.dma_start(out=at[:], in_=attr[:, g * GF * 4:(g + 1) * GF * 4])

                for t in range(GF):
                    gt = g * GF + t
                    usel = wp.tile([128, 16], F32, tag="usel")
                    nc.vector.select(out=usel[:], mask=ms[:, t:t + 1].to_broadcast([128, 16]),
                                     on_true=ug[:, t * 64 + 32:t * 64 + 48],
                                     on_false=ug[:, t * 64:t * 64 + 16])
                    vsel = wp.tile([128, 16], F32, tag="vsel")
                    nc.vector.select(out=vsel[:], mask=md[:, t:t + 1].to_broadcast([128, 16]),
                                     on_true=vg[:, t * 64 + 32:t * 64 + 48],
                                     on_false=vg[:, t * 64:t * 64 + 16])
                    m1 = wp.tile([128, 16], F32, tag="m1")
                    nc.vector.tensor_add(out=m1[:], in0=usel[:], in1=vsel[:])
                    for j in range(4):
                        nc.vector.scalar_tensor_tensor(
                            out=m1[:], in0=c_cb[:, j * 16:(j + 1) * 16],
                            scalar=at[:, t * 4 + j:t * 4 + j + 1],
                            in1=m1[:], op0=OP.mult, op1=OP.add)
                    rr = wp.tile([128, 16], F32, tag="rr")
                    nc.scalar.activation(rr[:], m1[:], AF.Relu)
                    rw = wp.tile([128, 16], F32, tag="rw")
                    nc.vector.tensor_tensor(out=rw[:], in0=rr[:], in1=c_w2b[:], op=OP.mult)
                    nc.vector.tensor_reduce(
                        out=fout[:, gt % 128:gt % 128 + 1], in_=rw[:], axis=AX.X, op=OP.add)

                if (g + 1) % (128 // GF) == 0 or g == NGF - 1:
                    fo2 = wp.tile([128, 128], F32, tag="fo2")
                    nc.vector.tensor_scalar(out=fo2[:], in0=fout[:], scalar1=FC2B,
                                            scalar2=None, op0=OP.add)
                    ft_ps = psA.tile([128, 128], F32, tag="a")
                    nc.tensor.transpose(out=ft_ps[:], in_=fo2[:], identity=c_idf[:])
                    ft = wp.tile([128, 128], F32, tag="ft_sb")
                    nc.vector.tensor_copy(out=ft[:], in_=ft_ps[:])
                    o0 = nsup * 16384
                    nc.sync.dma_start(
                        out=out_final[o0:o0 + 16384].rearrange("(a b) -> a b", b=128),
                        in_=ft[:])
                    nsup += 1
                    if g != NGF - 1:
                        fout = sp.tile([128, 128], F32, tag="fout")

    return nc


def kernel(**inputs):
    inputs = {k: np.asarray(v) for k, v in inputs.items()}
    in_maps, meta = _host_prep(**inputs)
    nc = _build(meta)
    nc.finalize()
    from concourse.bass_utils import run_bass_kernel_spmd
    res = run_bass_kernel_spmd(nc, in_maps, list(range(NCORES))).results
    EC = meta['EC']
    out = np.concatenate([res[k]["out_final"][:EC] for k in range(NCORES)])
    return out[:E].astype(np.float32)



# revision 11
# speedup vs baseline: 1.0447x; 1.0447x over previous
"""GAT (3-layer) + edge MLP on 8 TRN2 NeuronCores.

Strategy: dst-sorted edge sharding into per-core node slices; greedy
<=128-node blocks with fixed TLO+THI (lo/hi table half) 128-edge tiles;
segment softmax-sum via indicator-matrix matmuls accumulated in PSUM; bf16
tables gathered with int16 dma_gather; three small AllGathers between layers.

v2: tensor_scalar 4x paths (bf16 is_equal, per-head weighting, fast PSUM
copies), acc-style layer-1 aggregation (single weighted-aggregate matmul
per tile instead of a per-head pass), ACT-engine ELU decomposition, and no
redundant den matmuls on L2/L3 (den read from acc's appended ex columns).
"""
import sys
sys.path.insert(0, '/opt/trn_rl_repo')
import numpy as np
import ml_dtypes

BF = ml_dtypes.bfloat16

F_IN = 128
NEG = 0.2

N = 50000
E = 400000
NCORES = 8
SL = 6256
NP = SL * NCORES
HALF = NP // 2
TLO = 5
THI = 5
TT = TLO + THI
G = 4                    # blocks per gather group
GF = 4                   # final-stage tiles per gather group


def _wrap16(idx, pad=0):
    idx = np.asarray(idx, np.int64)
    n = len(idx)
    cols = (n + 15) // 16
    a = np.full((16, cols), pad, np.int16)
    a[np.arange(n) % 16, np.arange(n) // 16] = idx
    return np.tile(a, (8, 1))


def _bcast_rows(v, parts=128):
    v = np.asarray(v, np.float32).reshape(-1)
    return np.broadcast_to(v, (parts, v.size)).copy()


def _host_prep(x, edge_index, edge_attr, year, quarter,
               W1, a1s, a1d, b1, W2, a2s, a2d, b2, W3, a3s, a3d, b3,
               fc1_w, fc1_b, fc2_w, fc2_b):
    n = x.shape[0]
    loops = np.arange(n, dtype=np.int64)
    src = np.concatenate([edge_index[0].astype(np.int64), loops])
    dst = np.concatenate([edge_index[1].astype(np.int64), loops])
    order = np.argsort(dst, kind='stable')
    src_s, dst_s = src[order], dst[order]

    counts = np.bincount(dst_s, minlength=NP)
    starts = np.zeros(NP + 1, np.int64)
    np.cumsum(counts, out=starts[1:])

    per_core_blocks = []
    for k in range(NCORES):
        blocks = []
        node = k * SL
        end = min((k + 1) * SL, n)
        while node < end:
            bs = node
            lo_e, hi_e, dl_lo, dl_hi = [], [], [], []
            cnt = 0
            while node < end and cnt < 128:
                e0, e1 = starts[node], starts[node + 1]
                seg = src_s[e0:e1]
                seg_lo = seg[seg < HALF]
                seg_hi = seg[seg >= HALF]
                if len(lo_e) + len(seg_lo) > TLO * 128 or \
                   len(hi_e) + len(seg_hi) > THI * 128:
                    break
                lo_e.extend(seg_lo.tolist())
                dl_lo.extend([node - bs] * len(seg_lo))
                hi_e.extend((seg_hi - HALF).tolist())
                dl_hi.extend([node - bs] * len(seg_hi))
                node += 1
                cnt += 1
            assert cnt > 0
            blocks.append((bs, cnt, lo_e, hi_e, dl_lo, dl_hi))
        per_core_blocks.append(blocks)

    Bmax = max(len(b) for b in per_core_blocks)
    B = ((Bmax + G - 1) // G) * G
    NG = B // G

    EC = E // NCORES
    TF = (EC + 127) // 128
    TFP = ((TF + GF - 1) // GF) * GF
    NGF = TFP // GF
    OUTN = ((TFP * 128 + 16383) // 16384) * 16384

    H1, C1 = a1s.shape
    H2, C2 = a2s.shape
    W1r = W1.reshape(H1, C1, F_IN)
    w1t_packed = np.stack([W1r[h].T for h in range(H1)], 0)      # [4,128,128]
    w1s = np.einsum('hc,hcf->fh', a1s, W1r)
    w1d = np.einsum('hc,hcf->fh', a1d, W1r)
    shared = dict(
        w1t=np.concatenate([w1t_packed[h] for h in range(H1)], 1).astype(BF),
        w1s_bc=_bcast_rows(w1s.T.reshape(-1)).astype(BF),
        w1d_bc=_bcast_rows(w1d.T.reshape(-1)).astype(BF),
        w2t=W2.T.reshape(4, 128, H2 * C2).transpose(1, 0, 2).reshape(128, 512).astype(BF),
        a2s_bc=_bcast_rows(a2s.reshape(-1)).astype(BF),
        a2d_bc=_bcast_rows(a2d.reshape(-1)).astype(BF),
        w3t=W3.T.astype(BF),
        w3s_bc=_bcast_rows(a3s.reshape(-1)).astype(BF),
        w3d_bc=_bcast_rows(a3d.reshape(-1)).astype(BF),
        ab=np.concatenate([fc1_w[:, 0:8].T, fc1_w[:, 8:16].T], 1).astype(np.float32),
        cb=_bcast_rows(np.concatenate([fc1_w[:, 16], fc1_w[:, 17],
                                       fc1_w[:, 18], fc1_b])),
        w2b=_bcast_rows(fc2_w.reshape(-1)),
        b1_bc=_bcast_rows(b1), b2_bc=_bcast_rows(b2), b3_bc=_bcast_rows(b3),
        iota_bf=np.broadcast_to(np.arange(128).astype(BF), (128, 128)).copy(),
        ident_bf=np.eye(128, dtype=BF),
        ident_f=np.eye(128, dtype=np.float32),
    )
    x_pad = np.zeros((NP, F_IN), np.float32)
    x_pad[:n] = x
    x_bf = x_pad.astype(BF)

    in_maps = []
    for k in range(NCORES):
        blocks = per_core_blocks[k]
        lo_idx = np.zeros((NG, 128, G * TLO * 8), np.int16)
        hi_idx = np.zeros((NG, 128, G * THI * 8), np.int16)
        dstloc = np.full((128, B * TT), -1.0, np.float32)
        rows_all = np.full((B, 128), SL, np.int64)
        for b in range(B):
            if b < len(blocks):
                bs, cnt, lo_e, hi_e, dl_lo, dl_hi = blocks[b]
                for i, d in enumerate(dl_lo):
                    dstloc[i % 128, b * TT + i // 128] = d
                for i, d in enumerate(dl_hi):
                    j = TLO * 128 + i
                    dstloc[j % 128, b * TT + j // 128] = d
                rows_all[b, :cnt] = np.arange(bs, bs + cnt) - k * SL
        for g in range(NG):
            lo_cat, hi_cat, row_cat = [], [], []
            for b in range(g * G, (g + 1) * G):
                le = blocks[b][2] if b < len(blocks) else []
                he = blocks[b][3] if b < len(blocks) else []
                lo_cat.extend(le + [0] * (TLO * 128 - len(le)))
                hi_cat.extend(he + [0] * (THI * 128 - len(he)))
                row_cat.extend(rows_all[b].tolist())
            lo_idx[g] = _wrap16(lo_cat)
            hi_idx[g] = _wrap16(hi_cat)
            if g == 0:
                blkrow_idx = np.zeros((NG, 128, G * 8), np.int16)
            blkrow_idx[g] = _wrap16(row_cat)

        # per-block start row (for plain-DMA block-row reads); padded blocks
        # point at the zero row region past SL
        blk_bs = [blocks[b][0] - k * SL if b < len(blocks) else SL
                  for b in range(B)]

        fe0 = k * EC
        fsrc_p = np.zeros(TFP * 128, np.int64)
        fdst_p = np.zeros(TFP * 128, np.int64)
        fsrc_p[:EC] = src[fe0:fe0 + EC]
        fdst_p[:EC] = dst[fe0:fe0 + EC]
        fsrc_q = np.zeros((NGF, 128, GF * 8), np.int16)
        fdst_q = np.zeros((NGF, 128, GF * 8), np.int16)
        for g in range(NGF):
            s0 = g * GF * 128
            fsrc_q[g] = _wrap16(fsrc_p[s0:s0 + GF * 128] >> 1)
            fdst_q[g] = _wrap16(fdst_p[s0:s0 + GF * 128] >> 1)
        msrc = (fsrc_p & 1).astype(np.int8).reshape(TFP, 128).T.copy()
        mdst = (fdst_p & 1).astype(np.int8).reshape(TFP, 128).T.copy()
        att = np.zeros((TFP * 128, 4), np.float32)
        att[:EC, 0] = edge_attr.reshape(-1)[fe0:fe0 + EC]
        att[:EC, 1] = year.reshape(-1)[fe0:fe0 + EC]
        att[:EC, 2] = quarter.reshape(-1)[fe0:fe0 + EC]
        att[:EC, 3] = 1.0
        att = att.reshape(TFP, 128, 4).transpose(1, 0, 2).reshape(128, TFP * 4).copy()

        m = dict(shared)
        m.update(
            x_slice=np.vstack([x_bf[k * SL:(k + 1) * SL],
                               np.zeros((128, F_IN), BF)]),
            lo_idx=lo_idx, hi_idx=hi_idx, dstloc=dstloc, blkrow=blkrow_idx,
            fsrc=fsrc_q, fdst=fdst_q, msrc=msrc, mdst=mdst, attr=att,
        )
        in_maps.append(m)

    meta = dict(B=B, NG=NG, TFP=TFP, NGF=NGF, OUTN=OUTN, EC=EC,
                blk_bs=[[per_core_blocks[k][b][0] - k * SL
                         if b < len(per_core_blocks[k]) else SL
                         for b in range(B)] for k in range(NCORES)],
                has_bias=bool(np.abs(b1).max() > 0 or np.abs(b2).max() > 0
                              or np.abs(b3).max() > 0),
                fc2_b=float(np.asarray(fc2_b).reshape(())))
    return in_maps, meta


def _build(meta):
    from concourse import bass, bacc, mybir
    import concourse.tile as tile
    F32 = mybir.dt.float32
    BF16 = mybir.dt.bfloat16
    I16 = mybir.dt.int16
    AF = mybir.ActivationFunctionType
    OP = mybir.AluOpType
    AX = mybir.AxisListType

    B, NG, TFP, NGF, OUTN = (meta['B'], meta['NG'], meta['TFP'],
                             meta['NGF'], meta['OUTN'])
    FC2B = meta['fc2_b']
    HAS_BIAS = meta['has_bias']

    nc = bacc.Bacc(None, num_devices=NCORES, target_bir_lowering=False)
    P = lambda name, shape, dt: nc.declare_dram_parameter(name, shape, dt, isOutput=False)

    x_slice = P("x_slice", [SL + 128, 128], BF16)
    w1t = P("w1t", [128, 512], BF16)
    w1s_bc = P("w1s_bc", [128, 512], BF16)
    w1d_bc = P("w1d_bc", [128, 512], BF16)
    w2t = P("w2t", [128, 512], BF16)
    a2s_bc = P("a2s_bc", [128, 128], BF16)
    a2d_bc = P("a2d_bc", [128, 128], BF16)
    w3t = P("w3t", [128, 8], BF16)
    w3s_bc = P("w3s_bc", [128, 8], BF16)
    w3d_bc = P("w3d_bc", [128, 8], BF16)
    ab = P("ab", [8, 32], F32)
    cb = P("cb", [128, 64], F32)
    w2b = P("w2b", [128, 16], F32)
    b1_bc = P("b1_bc", [128, 512], F32)
    b2_bc = P("b2_bc", [128, 128], F32)
    b3_bc = P("b3_bc", [128, 8], F32)
    iota_bf_in = P("iota_bf", [128, 128], BF16)
    ident_bf_in = P("ident_bf", [128, 128], BF16)
    ident_f_in = P("ident_f", [128, 128], F32)
    lo_idx = P("lo_idx", [NG, 128, G * TLO * 8], I16)
    hi_idx = P("hi_idx", [NG, 128, G * THI * 8], I16)
    dstloc = P("dstloc", [128, B * TT], F32)
    blkrow = P("blkrow", [NG, 128, G * 8], I16)
    fsrc = P("fsrc", [NGF, 128, GF * 8], I16)
    fdst = P("fdst", [NGF, 128, GF * 8], I16)
    msrc = P("msrc", [128, TFP], mybir.dt.int8)
    mdst = P("mdst", [128, TFP], mybir.dt.int8)
    attr = P("attr", [128, TFP * 4], F32)

    out_final = nc.declare_dram_parameter("out_final", [OUTN], F32, isOutput=True)

    x_bounce = nc.dram_tensor("x_bounce", [SL, 128], BF16)
    h2_slice = nc.dram_tensor("h2_slice", [SL + 128, 128], BF16)
    h3_slice = nc.dram_tensor("h3_slice", [SL + 128, 128], BF16)
    h3o_slice = nc.dram_tensor("h3o_slice", [SL + 128, 64], F32)
    h3_comp = nc.dram_tensor("h3_comp", [SL, 8], BF16)
    h3o_comp = nc.dram_tensor("h3o_comp", [SL, 8], F32)
    t_x = nc.dram_tensor("t_x", [NP, 128], BF16, addr_space="Shared")
    t_h2 = nc.dram_tensor("t_h2", [NP, 128], BF16, addr_space="Shared")
    h3_ag = nc.dram_tensor("h3_ag", [NP, 8], BF16, addr_space="Shared")
    h3o_ag = nc.dram_tensor("h3o_ag", [NP, 8], F32, addr_space="Shared")
    t_h3 = nc.dram_tensor("t_h3", [NP, 128], BF16)
    t_u = nc.dram_tensor("t_u", [NP // 2, 64], F32)
    t_v = nc.dram_tensor("t_v", [NP // 2, 64], F32)

    with tile.TileContext(nc) as tc:
        with tc.tile_pool(name="const", bufs=1) as cp, \
             tc.tile_pool(name="stage", bufs=2) as sp, \
             tc.tile_pool(name="work", bufs=2) as wp, \
             tc.tile_pool(name="psA", bufs=2, space="PSUM") as psA, \
             tc.tile_pool(name="psB", bufs=2, space="PSUM") as psB, \
             tc.tile_pool(name="psAcc", bufs=1, space="PSUM") as psAcc:

            _cn = [0]
            def load_const(ap, shape, dt):
                _cn[0] += 1
                t = cp.tile(shape, dt, tag=f"const{_cn[0]}")
                nc.sync.dma_start(out=t[:], in_=ap)
                return t

            c_w1t = load_const(w1t[:, :], [128, 512], BF16)
            c_w1s = load_const(w1s_bc[:, :], [128, 512], BF16)
            c_w1d = load_const(w1d_bc[:, :], [128, 512], BF16)
            c_w2t = load_const(w2t[:, :], [128, 512], BF16)
            c_a2s = load_const(a2s_bc[:, :], [128, 128], BF16)
            c_a2d = load_const(a2d_bc[:, :], [128, 128], BF16)
            c_w3t = load_const(w3t[:, :], [128, 8], BF16)
            c_w3s = load_const(w3s_bc[:, :], [128, 8], BF16)
            c_w3d = load_const(w3d_bc[:, :], [128, 8], BF16)
            c_ab = load_const(ab[:, :], [8, 32], F32)
            c_cb = load_const(cb[:, :], [128, 64], F32)
            c_w2b = load_const(w2b[:, :], [128, 16], F32)
            c_b1 = load_const(b1_bc[:, :], [128, 512], F32)
            c_b2 = load_const(b2_bc[:, :], [128, 128], F32)
            c_b3 = load_const(b3_bc[:, :], [128, 8], F32)
            c_iota = load_const(iota_bf_in[:, :], [128, 128], BF16)
            c_idbf = load_const(ident_bf_in[:, :], [128, 128], BF16)
            c_idf = load_const(ident_f_in[:, :], [128, 128], F32)

            # zero scatter-target slices
            zt = cp.tile([128, 128], F32)
            nc.vector.memset(zt[:], 0.0)
            for tbl, wcols in ((h2_slice, 64), (h3_slice, 64), (h3o_slice, 64)):
                view = tbl[:, :].bitcast(F32)
                rows = SL + 128
                for r0 in range(0, rows, 128):
                    r1 = min(r0 + 128, rows)
                    nc.sync.dma_start(out=view[r0:r1, :], in_=zt[:r1 - r0, :wcols])

            def ts_copy(dst_ap, src_ap, tag=None, shape=None, dt=None):
                """PSUM->SBUF copy via tensor_scalar mult-by-1 (2x for bf16)."""
                nc.vector.tensor_scalar(out=dst_ap, in0=src_ap, scalar1=1.0,
                                        scalar2=None, op0=OP.mult)

            def elu_act(m_ps, bias_ap, width, out_dt):
                """ELU from PSUM pre-activation: 2 ACT + 1 DVE max + 1 DVE stt.
                m may have bias pre-added (HAS_BIAS path adds first)."""
                if HAS_BIAS:
                    mb = wp.tile([128, width], F32, tag=f"elmb{width}")
                    nc.vector.tensor_add(out=mb[:], in0=m_ps, in1=bias_ap)
                    m_ap = mb[:]
                else:
                    m_ap = m_ps
                r1 = wp.tile([128, width], F32, tag=f"elr1{width}")
                nc.scalar.activation(r1[:], m_ap, AF.Relu, scale=-1.0)
                e = wp.tile([128, width], F32, tag=f"ele{width}")
                nc.scalar.activation(e[:], r1[:], AF.Exp, scale=-1.0)
                r = wp.tile([128, width], F32, tag=f"elr{width}")
                nc.vector.tensor_scalar(out=r[:], in0=m_ap, scalar1=0.0,
                                        scalar2=None, op0=OP.max)
                out = wp.tile([128, width], out_dt, tag=f"elo{width}")
                nc.vector.scalar_tensor_tensor(
                    out=out[:], in0=e[:], scalar=-1.0, in1=r[:],
                    op0=OP.add, op1=OP.add)
                return out

            def edge_layer(layer, src_tbl, blk_tbl, scat_tbl, scat_elem):
                """Emit one GAT layer. Returns nothing; writes scat_tbl."""
                nH = 4 if layer != 3 else 1
                for g in range(NG):
                    li = sp.tile([128, G * TLO * 8], I16, tag="li")
                    nc.sync.dma_start(out=li[:], in_=lo_idx[g])
                    hi = sp.tile([128, G * THI * 8], I16, tag="hi")
                    nc.sync.dma_start(out=hi[:], in_=hi_idx[g])
                    bi = sp.tile([128, G * 8], I16, tag="bi")
                    nc.sync.dma_start(out=bi[:], in_=blkrow[g])
                    dl = sp.tile([128, G * TT], F32, tag="dl")
                    nc.sync.dma_start(out=dl[:], in_=dstloc[:, g * G * TT:(g + 1) * G * TT])
                    stag_lo = sp.tile([128, G * TLO * 128], BF16, tag="stag_lo")
                    stag_hi = sp.tile([128, G * THI * 128], BF16, tag="stag_hi")
                    for bg in range(G):
                        nc.gpsimd.dma_gather(
                            out_ap=stag_lo[:, bg * TLO * 128:(bg + 1) * TLO * 128]
                                .rearrange("p (a b) -> p a b", a=TLO),
                            in_ap=src_tbl[0:HALF, :],
                            idxs_ap=li[:, bg * TLO * 8:(bg + 1) * TLO * 8],
                            num_idxs=TLO * 128, num_idxs_reg=TLO * 128,
                            elem_size=128)
                        nc.gpsimd.dma_gather(
                            out_ap=stag_hi[:, bg * THI * 128:(bg + 1) * THI * 128]
                                .rearrange("p (a b) -> p a b", a=THI),
                            in_ap=src_tbl[HALF:NP, :],
                            idxs_ap=hi[:, bg * THI * 8:(bg + 1) * THI * 8],
                            num_idxs=THI * 128, num_idxs_reg=THI * 128,
                            elem_size=128)
                    brow = sp.tile([128, G * 128], BF16, tag="brow")
                    nc.gpsimd.dma_gather(
                        out_ap=brow[:].rearrange("p (a b) -> p a b", a=G),
                        in_ap=blk_tbl[:, :], idxs_ap=bi[:],
                        num_idxs=G * 128, num_idxs_reg=G * 128, elem_size=128)

                    owid = 128 if layer != 3 else 64
                    ostage = sp.tile([128, G * owid],
                                     BF16 if layer != 3 else F32, tag="ost")

                    for bg in range(G):
                        xb = brow[:, bg * 128:(bg + 1) * 128]
                        # --- d per block node (fused dot products) ---
                        dblk = wp.tile([128, nH], F32, tag="dblk")
                        if layer == 1:
                            tmp = wp.tile([128, 512], F32, tag="dtmp")
                            nc.vector.tensor_tensor(
                                out=tmp[:],
                                in0=xb.rearrange("p (o f) -> p o f", o=1).to_broadcast([128, 4, 128]),
                                in1=c_w1d[:].rearrange("p (h f) -> p h f", h=4),
                                op=OP.mult)
                            nc.vector.tensor_reduce(
                                out=dblk[:], in_=tmp[:].rearrange("p (h f) -> p h f", h=4),
                                axis=AX.X, op=OP.add)
                        elif layer == 2:
                            tmp = wp.tile([128, 128], F32, tag="dtmp2")
                            nc.vector.tensor_tensor(out=tmp[:], in0=xb, in1=c_a2d[:], op=OP.mult)
                            nc.vector.tensor_reduce(
                                out=dblk[:], in_=tmp[:].rearrange("p (h f) -> p h f", h=4),
                                axis=AX.X, op=OP.add)
                        else:
                            tmp = wp.tile([128, 8], F32, tag="dtmp3")
                            nc.vector.tensor_tensor(out=tmp[:], in0=xb[:, 0:8], in1=c_w3d[:], op=OP.mult)
                            nc.vector.tensor_reduce(out=dblk[:], in_=tmp[:], axis=AX.X, op=OP.add)
                        dblk_bf = wp.tile([128, nH], BF16, tag="dblk_bf")
                        nc.vector.tensor_copy(out=dblk_bf[:], in_=dblk[:])

                        # --- pass 0 over tiles: m0 cache, exc (bf16), den/acc ---
                        m0c = wp.tile([128, TT * 128], BF16, tag="m0c")
                        exc = wp.tile([128, TT * nH], F32, tag="exc")
                        if layer == 1:
                            den_ps = psAcc.tile([128, nH], F32, tag="den")
                            acc_ps = psAcc.tile([128, 512], F32, tag="acc")
                        else:
                            num_w = 132 if layer == 2 else 9
                            acc_ps = psAcc.tile([128, num_w], F32, tag="acc")
                        for t in range(TT):
                            sl0 = (bg * TLO + t) * 128 if t < TLO else (bg * THI + (t - TLO)) * 128
                            xg = (stag_lo if t < TLO else stag_hi)[:, sl0:sl0 + 128]
                            dcol = dl[:, bg * TT + t:bg * TT + t + 1]
                            m0 = m0c[:, t * 128:(t + 1) * 128]
                            nc.vector.tensor_scalar(
                                out=m0, in0=c_iota[:], scalar1=dcol,
                                scalar2=None, op0=OP.is_equal)
                            m0t_ps = psB.tile([128, 128], BF16, tag="b")
                            nc.tensor.transpose(out=m0t_ps[:], in_=m0, identity=c_idbf[:])
                            m0t = wp.tile([128, 128], BF16, tag="m0t_sb")
                            ts_copy(m0t[:], m0t_ps[:], "m0t", [128, 128], BF16)
                            de_ps = psA.tile([128, nH], F32, tag="a")
                            nc.tensor.matmul(de_ps[:], m0t[:], dblk_bf[:], start=True, stop=True)
                            sg = wp.tile([128, nH], F32, tag="sg")
                            if layer == 1:
                                tmp2 = wp.tile([128, 512], F32, tag="stmp")
                                nc.vector.tensor_tensor(
                                    out=tmp2[:],
                                    in0=xg.rearrange("p (o f) -> p o f", o=1).to_broadcast([128, 4, 128]),
                                    in1=c_w1s[:].rearrange("p (h f) -> p h f", h=4), op=OP.mult)
                                nc.vector.tensor_reduce(
                                    out=sg[:], in_=tmp2[:].rearrange("p (h f) -> p h f", h=4),
                                    axis=AX.X, op=OP.add)
                            elif layer == 2:
                                tmp2 = wp.tile([128, 128], F32, tag="stmp2")
                                nc.vector.tensor_tensor(out=tmp2[:], in0=xg, in1=c_a2s[:], op=OP.mult)
                                nc.vector.tensor_reduce(
                                    out=sg[:], in_=tmp2[:].rearrange("p (h f) -> p h f", h=4),
                                    axis=AX.X, op=OP.add)
                            else:
                                tmp2 = wp.tile([128, 8], F32, tag="stmp3")
                                nc.vector.tensor_tensor(out=tmp2[:], in0=xg[:, 0:8], in1=c_w3s[:], op=OP.mult)
                                nc.vector.tensor_reduce(out=sg[:], in_=tmp2[:], axis=AX.X, op=OP.add)
                            raw = wp.tile([128, nH], F32, tag="raw")
                            nc.vector.tensor_add(out=raw[:], in0=sg[:], in1=de_ps[:])
                            lr = wp.tile([128, nH], F32, tag="lr")
                            nc.vector.scalar_tensor_tensor(
                                out=lr[:], in0=raw[:], scalar=NEG, in1=raw[:],
                                op0=OP.mult, op1=OP.max)
                            exd = exc[:, t * nH:(t + 1) * nH]
                            nc.scalar.activation(exd, lr[:], AF.Exp)
                            ex_bf = wp.tile([128, nH], BF16, tag="ex_bf")
                            nc.vector.tensor_copy(out=ex_bf[:], in_=exd)
                            first, last = (t == 0), (t == TT - 1)
                            if layer == 1:
                                nc.tensor.matmul(den_ps[:], m0, ex_bf[:], start=first, stop=last)
                                gw = wp.tile([128, 512], BF16, tag="gw")
                                for h in range(4):
                                    nc.vector.tensor_scalar(
                                        out=gw[:, h * 128:(h + 1) * 128], in0=xg,
                                        scalar1=exd[:, h:h + 1], scalar2=None,
                                        op0=OP.mult)
                                nc.tensor.matmul(acc_ps[:], m0, gw[:], start=first, stop=last)
                            else:
                                gw = wp.tile([128, num_w], BF16, tag="gw")
                                if layer == 2:
                                    for h in range(4):
                                        nc.vector.tensor_scalar(
                                            out=gw[:, h * 32:(h + 1) * 32],
                                            in0=xg[:, h * 32:(h + 1) * 32],
                                            scalar1=exd[:, h:h + 1], scalar2=None,
                                            op0=OP.mult)
                                    nc.vector.tensor_copy(out=gw[:, 128:132], in_=ex_bf[:])
                                else:
                                    nc.vector.tensor_scalar(
                                        out=gw[:, 0:8], in0=xg[:, 0:8],
                                        scalar1=exd[:, 0:1], scalar2=None, op0=OP.mult)
                                    nc.vector.tensor_copy(out=gw[:, 8:9], in_=ex_bf[:])
                                nc.tensor.matmul(acc_ps[:], m0, gw[:], start=first, stop=last)

                        if layer == 1:
                            den_ap = den_ps[:]
                        elif layer == 2:
                            den_ap = acc_ps[:, 128:132]
                        else:
                            den_ap = acc_ps[:, 8:9]
                        den_sb = wp.tile([128, nH], F32, tag="den_sb")
                        nc.vector.tensor_scalar(out=den_sb[:], in0=den_ap, scalar1=1e-30,
                                                 scalar2=None, op0=OP.max)
                        rec = wp.tile([128, nH], F32, tag="rec")
                        nc.vector.reciprocal(out=rec[:], in_=den_sb[:])

                        if layer == 1:
                            # normalize + transform: nh = (acc/den) @ w1t per head
                            nh_ps = psA.tile([128, 512], F32, tag="nh")
                            for h in range(4):
                                pc = wp.tile([128, 128], BF16, tag="pc")
                                nc.vector.tensor_scalar(
                                    out=pc[:], in0=acc_ps[:, h * 128:(h + 1) * 128],
                                    scalar1=rec[:, h:h + 1], scalar2=None, op0=OP.mult)
                                pt_ps = psB.tile([128, 128], BF16, tag="b")
                                nc.tensor.transpose(out=pt_ps[:], in_=pc[:], identity=c_idbf[:])
                                pt = wp.tile([128, 128], BF16, tag="pt_sb")
                                ts_copy(pt[:], pt_ps[:], "pt", None, None)
                                nc.tensor.matmul(nh_ps[:, h * 128:(h + 1) * 128],
                                                 pt[:], c_w1t[:, h * 128:(h + 1) * 128],
                                                 start=True, stop=True)
                            elu1 = elu_act(nh_ps[:], c_b1[:], 512, BF16)
                            # dense2 -> h2 block
                            h2_ps = psA.tile([128, 128], F32, tag="a")
                            for c in range(4):
                                ct_ps = psB.tile([128, 128], BF16, tag="b")
                                nc.tensor.transpose(out=ct_ps[:], in_=elu1[:, c * 128:(c + 1) * 128],
                                                    identity=c_idbf[:])
                                ct = wp.tile([128, 128], BF16, tag="ct_sb")
                                ts_copy(ct[:], ct_ps[:], "ct", None, None)
                                nc.tensor.matmul(h2_ps[:], ct[:], c_w2t[:, c * 128:(c + 1) * 128],
                                                 start=(c == 0), stop=(c == 3))
                            nc.vector.tensor_copy(out=ostage[:, bg * 128:(bg + 1) * 128],
                                                  in_=h2_ps[:])
                        elif layer == 2:
                            o_sb = wp.tile([128, 128], F32, tag="o_sb2")
                            for h in range(4):
                                nc.vector.tensor_scalar(
                                    out=o_sb[:, h * 32:(h + 1) * 32],
                                    in0=acc_ps[:, h * 32:(h + 1) * 32],
                                    scalar1=rec[:, h:h + 1], scalar2=None, op0=OP.mult)
                            elu2 = elu_act(o_sb[:], c_b2[:], 128, BF16)
                            ct_ps = psB.tile([128, 128], BF16, tag="b")
                            nc.tensor.transpose(out=ct_ps[:], in_=elu2[:], identity=c_idbf[:])
                            ct = wp.tile([128, 128], BF16, tag="ct_sb")
                            ts_copy(ct[:], ct_ps[:], "ct2", None, None)
                            h3_ps = psA.tile([128, 8], F32, tag="a")
                            nc.tensor.matmul(h3_ps[:], ct[:], c_w3t[:], start=True, stop=True)
                            st = ostage[:, bg * 128:(bg + 1) * 128]
                            nc.vector.memset(st, 0.0)
                            nc.vector.tensor_copy(out=ostage[:, bg * 128:bg * 128 + 8],
                                                  in_=h3_ps[:])
                        else:
                            o_sb = wp.tile([128, 8], F32, tag="o_sb3")
                            nc.vector.tensor_scalar(
                                out=o_sb[:], in0=acc_ps[:, 0:8],
                                scalar1=rec[:, 0:1], scalar2=None, op0=OP.mult)
                            elu3 = elu_act(o_sb[:], c_b3[:], 8, F32)
                            st = ostage[:, bg * 64:(bg + 1) * 64]
                            nc.vector.memset(st, 0.0)
                            nc.vector.tensor_copy(out=ostage[:, bg * 64:bg * 64 + 8],
                                                  in_=elu3[:])

                    nc.gpsimd.dma_scatter_add(
                        scat_tbl[:, :], ostage[:].rearrange("p (a b) -> p a b", a=G),
                        bi[:], G * 128, G * 128, scat_elem)

            # ======== layers ========
            nc.sync.dma_start(out=x_bounce[:, :], in_=x_slice[0:SL, :])
            nc.gpsimd.collective_compute(
                "AllGather", mybir.AluOpType.bypass,
                replica_groups=[list(range(NCORES))],
                ins=[x_bounce[:, :]], outs=[t_x[:, :]])
            edge_layer(1, t_x, x_slice, h2_slice, 128)
            nc.gpsimd.collective_compute(
                "AllGather", mybir.AluOpType.bypass,
                replica_groups=[list(range(NCORES))],
                ins=[h2_slice[0:SL, :]], outs=[t_h2[:, :]])

            edge_layer(2, t_h2, h2_slice, h3_slice, 128)
            for r0 in range(0, SL, 512):
                r1 = min(r0 + 512, SL)
                nc.sync.dma_start(out=h3_comp[r0:r1, :],
                                  in_=h3_slice[r0:r1, 0:8])
            nc.gpsimd.collective_compute(
                "AllGather", mybir.AluOpType.bypass,
                replica_groups=[list(range(NCORES))],
                ins=[h3_comp[:, :]], outs=[h3_ag[:, :]])
            for r0 in range(0, NP, 512):
                r1 = min(r0 + 512, NP)
                nc.sync.dma_start(out=t_h3[r0:r1, 0:8],
                                  in_=h3_ag[r0:r1, :])

            edge_layer(3, t_h3, h3_slice, h3o_slice, 64)
            for r0 in range(0, SL, 512):
                r1 = min(r0 + 512, SL)
                nc.sync.dma_start(out=h3o_comp[r0:r1, :],
                                  in_=h3o_slice[r0:r1, 0:8])
            nc.gpsimd.collective_compute(
                "AllGather", mybir.AluOpType.bypass,
                replica_groups=[list(range(NCORES))],
                ins=[h3o_comp[:, :]], outs=[h3o_ag[:, :]])

            # ======== u/v tables (pack-2 rows) ========
            for r in range(NP // 128):
                hrows = wp.tile([128, 8], F32, tag="hrows")
                nc.sync.dma_start(out=hrows[:], in_=h3o_ag[r * 128:(r + 1) * 128, :])
                ht_ps = psA.tile([128, 128], F32, tag="a")
                nc.tensor.transpose(out=ht_ps[0:8, :], in_=hrows[:], identity=c_idf[:])
                ht = wp.tile([8, 128], F32, tag="ht_sb")
                nc.vector.tensor_copy(out=ht[:], in_=ht_ps[0:8, :])
                uv_ps = psA.tile([128, 32], F32, tag="a")
                nc.tensor.matmul(uv_ps[:], ht[:], c_ab[:], start=True, stop=True)
                ust = wp.tile([128, 16], F32, tag="ust")
                nc.vector.tensor_copy(out=ust[:], in_=uv_ps[:, 0:16])
                vst = wp.tile([128, 16], F32, tag="vst")
                nc.vector.tensor_copy(out=vst[:], in_=uv_ps[:, 16:32])
                du = t_u[:, :].rearrange("r (s c) -> r s c", s=2)
                nc.sync.dma_start(out=du[r * 64:(r + 1) * 64, :, 0:16], in_=ust[:])
                dv = t_v[:, :].rearrange("r (s c) -> r s c", s=2)
                nc.sync.dma_start(out=dv[r * 64:(r + 1) * 64, :, 0:16], in_=vst[:])

            # ======== final edge MLP ========
            fout = sp.tile([128, 128], F32, tag="fout")
            nsup = 0
            for g in range(NGF):
                fi = sp.tile([128, GF * 8], I16, tag="fi")
                nc.sync.dma_start(out=fi[:], in_=fsrc[g])
                di = sp.tile([128, GF * 8], I16, tag="di")
                nc.sync.dma_start(out=di[:], in_=fdst[g])
                ug = sp.tile([128, GF * 64], F32, tag="ug")
                nc.gpsimd.dma_gather(
                    out_ap=ug[:].rearrange("p (a b) -> p a b", a=GF),
                    in_ap=t_u[:, :], idxs_ap=fi[:],
                    num_idxs=GF * 128, num_idxs_reg=GF * 128, elem_size=64)
                vg = sp.tile([128, GF * 64], F32, tag="vg")
                nc.gpsimd.dma_gather(
                    out_ap=vg[:].rearrange("p (a b) -> p a b", a=GF),
                    in_ap=t_v[:, :], idxs_ap=di[:],
                    num_idxs=GF * 128, num_idxs_reg=GF * 128, elem_size=64)
                ms = sp.tile([128, GF], mybir.dt.int8, tag="ms")
                nc.sync.dma_start(out=ms[:], in_=msrc[:, g * GF:(g + 1) * GF])
                md = sp.tile([128, GF], mybir.dt.int8, tag="md")
                nc.sync.dma_start(out=md[:], in_=mdst[:, g * GF:(g + 1) * GF])
                at = sp.tile([128, GF * 4], F32, tag="at")
                nc.sync.dma_start(out=at[:], in_=attr[:, g * GF * 4:(g + 1) * GF * 4])

                for t in range(GF):
                    gt = g * GF + t
                    usel = wp.tile([128, 16], F32, tag="usel")
                    nc.vector.select(out=usel[:], mask=ms[:, t:t + 1].to_broadcast([128, 16]),
                                     on_true=ug[:, t * 64 + 32:t * 64 + 48],
                                     on_false=ug[:, t * 64:t * 64 + 16])
                    vsel = wp.tile([128, 16], F32, tag="vsel")
                    nc.vector.select(out=vsel[:], mask=md[:, t:t + 1].to_broadcast([128, 16]),
                                     on_true=vg[:, t * 64 + 32:t * 64 + 48],
                                     on_false=vg[:, t * 64:t * 64 + 16])
                    m1 = wp.tile([128, 16], F32, tag="m1")
                    nc.vector.tensor_add(out=m1[:], in0=usel[:], in1=vsel[:])
                    for j in range(4):
                        nc.vector.scalar_tensor_tensor(
                            out=m1[:], in0=c_cb[:, j * 16:(j + 1) * 16],
                            scalar=at[:, t * 4 + j:t * 4 + j + 1],
                            in1=m1[:], op0=OP.mult, op1=OP.add)
                    rr = wp.tile([128, 16], F32, tag="rr")
                    nc.scalar.activation(rr[:], m1[:], AF.Relu)
                    rw = wp.tile([128, 16], F32, tag="rw")
                    nc.vector.tensor_tensor(out=rw[:], in0=rr[:], in1=c_w2b[:], op=OP.mult)
                    nc.vector.tensor_reduce(
                        out=fout[:, gt % 128:gt % 128 + 1], in_=rw[:], axis=AX.X, op=OP.add)

                if (g + 1) % (128 // GF) == 0 or g == NGF - 1:
                    fo2 = wp.tile([128, 128], F32, tag="fo2")
                    nc.vector.tensor_scalar(out=fo2[:], in0=fout[:], scalar1=FC2B,
                                            scalar2=None, op0=OP.add)
                    ft_ps = psA.tile([128, 128], F32, tag="a")
                    nc.tensor.transpose(out=ft_ps[:], in_=fo2[:], identity=c_idf[:])
                    ft = wp.tile([128, 128], F32, tag="ft_sb")
                    nc.vector.tensor_copy(out=ft[:], in_=ft_ps[:])
                    o0 = nsup * 16384
                    nc.sync.dma_start(
                        out=out_final[o0:o0 + 16384].rearrange("(a b) -> a b", b=128),
                        in_=ft[:])
                    nsup += 1
                    if g != NGF - 1:
                        fout = sp.tile([128, 128], F32, tag="fout")

    return nc


def kernel(**inputs):
    inputs = {k: np.asarray(v) for k, v in inputs.items()}
    in_maps, meta = _host_prep(**inputs)
    nc = _build(meta)
    nc.finalize()
    from concourse.bass_utils import run_bass_kernel_spmd
    res = run_bass_kernel_spmd(nc, in_maps, list(range(NCORES))).results
    EC = meta['EC']
    out = np.concatenate([res[k]["out_final"][:EC] for k in range(NCORES)])
    return out[:E].astype(np.float32)


# revision 14
# speedup vs baseline: 1.1766x; 1.1262x over previous
"""GAT (3-layer) + edge MLP on 8 TRN2 NeuronCores.

Strategy: dst-sorted edge sharding into per-core node slices; greedy
<=128-node blocks with fixed TLO+THI (lo/hi table half) 128-edge tiles;
segment softmax-sum via indicator-matrix matmuls accumulated in PSUM; bf16
tables gathered with int16 dma_gather; three small AllGathers between layers.

v2: tensor_scalar 4x paths (bf16 is_equal, per-head weighting, fast PSUM
copies), acc-style layer-1 aggregation (single weighted-aggregate matmul
per tile instead of a per-head pass), ACT-engine ELU decomposition, and no
redundant den matmuls on L2/L3 (den read from acc's appended ex columns).
"""
import sys
sys.path.insert(0, '/opt/trn_rl_repo')
import numpy as np
import ml_dtypes

BF = ml_dtypes.bfloat16

F_IN = 128
NEG = 0.2

N = 50000
E = 400000
NCORES = 8
SL = 6256
NP = SL * NCORES
HALF = NP // 2
TLO = 5
THI = 5
TT = TLO + THI
G = 4                    # blocks per gather group
GF = 4                   # final-stage tiles per gather group


def _wrap16(idx, pad=0):
    idx = np.asarray(idx, np.int64)
    n = len(idx)
    cols = (n + 15) // 16
    a = np.full((16, cols), pad, np.int16)
    a[np.arange(n) % 16, np.arange(n) // 16] = idx
    return np.tile(a, (8, 1))


def _blockdiag(a):
    # a [H, C] -> [H*C, H] with column h = a[h] in rows h*C:(h+1)*C
    H, C = a.shape
    m = np.zeros((H * C, H), np.float32)
    for h in range(H):
        m[h * C:(h + 1) * C, h] = a[h]
    return m


def _bcast_rows(v, parts=128):
    v = np.asarray(v, np.float32).reshape(-1)
    return np.broadcast_to(v, (parts, v.size)).copy()


def _host_prep(x, edge_index, edge_attr, year, quarter,
               W1, a1s, a1d, b1, W2, a2s, a2d, b2, W3, a3s, a3d, b3,
               fc1_w, fc1_b, fc2_w, fc2_b):
    n = x.shape[0]
    loops = np.arange(n, dtype=np.int64)
    src = np.concatenate([edge_index[0].astype(np.int64), loops])
    dst = np.concatenate([edge_index[1].astype(np.int64), loops])
    order = np.argsort(dst, kind='stable')
    src_s, dst_s = src[order], dst[order]

    counts = np.bincount(dst_s, minlength=NP)
    starts = np.zeros(NP + 1, np.int64)
    np.cumsum(counts, out=starts[1:])

    per_core_blocks = []
    for k in range(NCORES):
        blocks = []
        node = k * SL
        end = min((k + 1) * SL, n)
        while node < end:
            bs = node
            lo_e, hi_e, dl_lo, dl_hi = [], [], [], []
            cnt = 0
            while node < end and cnt < 128:
                e0, e1 = starts[node], starts[node + 1]
                seg = src_s[e0:e1]
                seg_lo = seg[seg < HALF]
                seg_hi = seg[seg >= HALF]
                if len(lo_e) + len(seg_lo) > TLO * 128 or \
                   len(hi_e) + len(seg_hi) > THI * 128:
                    break
                lo_e.extend(seg_lo.tolist())
                dl_lo.extend([node - bs] * len(seg_lo))
                hi_e.extend((seg_hi - HALF).tolist())
                dl_hi.extend([node - bs] * len(seg_hi))
                node += 1
                cnt += 1
            assert cnt > 0
            blocks.append((bs, cnt, lo_e, hi_e, dl_lo, dl_hi))
        per_core_blocks.append(blocks)

    Bmax = max(len(b) for b in per_core_blocks)
    B = ((Bmax + G - 1) // G) * G
    NG = B // G

    EC = E // NCORES
    TF = (EC + 127) // 128
    TFP = ((TF + GF - 1) // GF) * GF
    NGF = TFP // GF
    OUTN = ((TFP * 128 + 16383) // 16384) * 16384

    H1, C1 = a1s.shape
    H2, C2 = a2s.shape
    W1r = W1.reshape(H1, C1, F_IN)
    w1t_packed = np.stack([W1r[h].T for h in range(H1)], 0)      # [4,128,128]
    w1s = np.einsum('hc,hcf->fh', a1s, W1r)
    w1d = np.einsum('hc,hcf->fh', a1d, W1r)
    shared = dict(
        w1t=np.concatenate([w1t_packed[h] for h in range(H1)], 1).astype(BF),
        w2t=W2.T.reshape(4, 128, H2 * C2).transpose(1, 0, 2).reshape(128, 512).astype(BF),
        w3t=W3.T.astype(BF),
        wsd1=np.concatenate([w1s, w1d], 1).astype(BF),
        wsd2=np.concatenate([_blockdiag(a2s), _blockdiag(a2d)], 1).astype(BF),
        wsd3=np.concatenate([np.pad(a3s.reshape(8, 1), ((0, 120), (0, 0))),
                             np.pad(a3d.reshape(8, 1), ((0, 120), (0, 0)))], 1).astype(BF),
        ab=np.concatenate([fc1_w[:, 0:8].T, fc1_w[:, 8:16].T], 1).astype(np.float32),
        cb=_bcast_rows(np.concatenate([fc1_w[:, 16], fc1_w[:, 17],
                                       fc1_w[:, 18], fc1_b])),
        w2b=_bcast_rows(fc2_w.reshape(-1)),
        b1_bc=_bcast_rows(b1), b2_bc=_bcast_rows(b2), b3_bc=_bcast_rows(b3),
        iota_bf=np.broadcast_to(np.arange(128).astype(BF), (128, 128)).copy(),
        ident_bf=np.eye(128, dtype=BF),
        ident_f=np.eye(128, dtype=np.float32),
    )
    x_pad = np.zeros((NP, F_IN), np.float32)
    x_pad[:n] = x
    x_bf = x_pad.astype(BF)

    in_maps = []
    for k in range(NCORES):
        blocks = per_core_blocks[k]
        lo_idx = np.zeros((NG, 128, G * TLO * 8), np.int16)
        hi_idx = np.zeros((NG, 128, G * THI * 8), np.int16)
        dstloc = np.full((128, B * TT), -1.0, np.float32)
        rows_all = np.full((B, 128), SL, np.int64)
        for b in range(B):
            if b < len(blocks):
                bs, cnt, lo_e, hi_e, dl_lo, dl_hi = blocks[b]
                for i, d in enumerate(dl_lo):
                    dstloc[i % 128, b * TT + i // 128] = d
                for i, d in enumerate(dl_hi):
                    j = TLO * 128 + i
                    dstloc[j % 128, b * TT + j // 128] = d
                rows_all[b, :cnt] = np.arange(bs, bs + cnt) - k * SL
        for g in range(NG):
            lo_cat, hi_cat, row_cat = [], [], []
            for b in range(g * G, (g + 1) * G):
                le = blocks[b][2] if b < len(blocks) else []
                he = blocks[b][3] if b < len(blocks) else []
                lo_cat.extend(le + [0] * (TLO * 128 - len(le)))
                hi_cat.extend(he + [0] * (THI * 128 - len(he)))
                row_cat.extend(rows_all[b].tolist())
            lo_idx[g] = _wrap16(lo_cat)
            hi_idx[g] = _wrap16(hi_cat)
            if g == 0:
                blkrow_idx = np.zeros((NG, 128, G * 8), np.int16)
            blkrow_idx[g] = _wrap16(row_cat)

        # per-block start row (for plain-DMA block-row reads); padded blocks
        # point at the zero row region past SL
        blk_bs = [blocks[b][0] - k * SL if b < len(blocks) else SL
                  for b in range(B)]

        fe0 = k * EC
        fsrc_p = np.zeros(TFP * 128, np.int64)
        fdst_p = np.zeros(TFP * 128, np.int64)
        fsrc_p[:EC] = src[fe0:fe0 + EC]
        fdst_p[:EC] = dst[fe0:fe0 + EC]
        fsrc_q = np.zeros((NGF, 128, GF * 8), np.int16)
        fdst_q = np.zeros((NGF, 128, GF * 8), np.int16)
        for g in range(NGF):
            s0 = g * GF * 128
            fsrc_q[g] = _wrap16(fsrc_p[s0:s0 + GF * 128] >> 1)
            fdst_q[g] = _wrap16(fdst_p[s0:s0 + GF * 128] >> 1)
        msrc = (fsrc_p & 1).astype(np.int8).reshape(TFP, 128).T.copy()
        mdst = (fdst_p & 1).astype(np.int8).reshape(TFP, 128).T.copy()
        att = np.zeros((TFP * 128, 4), np.float32)
        att[:EC, 0] = edge_attr.reshape(-1)[fe0:fe0 + EC]
        att[:EC, 1] = year.reshape(-1)[fe0:fe0 + EC]
        att[:EC, 2] = quarter.reshape(-1)[fe0:fe0 + EC]
        att[:EC, 3] = 1.0
        att = att.reshape(TFP, 128, 4).transpose(1, 0, 2).reshape(128, TFP * 4).copy()

        m = dict(shared)
        m.update(
            x_slice=np.vstack([x_bf[k * SL:(k + 1) * SL],
                               np.zeros((128, F_IN), BF)]),
            lo_idx=lo_idx, hi_idx=hi_idx, dstloc=dstloc, blkrow=blkrow_idx,
            fsrc=fsrc_q, fdst=fdst_q, msrc=msrc, mdst=mdst, attr=att,
        )
        in_maps.append(m)

    meta = dict(B=B, NG=NG, TFP=TFP, NGF=NGF, OUTN=OUTN, EC=EC,
                blk_bs=[[per_core_blocks[k][b][0] - k * SL
                         if b < len(per_core_blocks[k]) else SL
                         for b in range(B)] for k in range(NCORES)],
                has_bias=bool(np.abs(b1).max() > 0 or np.abs(b2).max() > 0
                              or np.abs(b3).max() > 0),
                fc2_b=float(np.asarray(fc2_b).reshape(())))
    return in_maps, meta


def _build(meta):
    from concourse import bass, bacc, mybir
    import concourse.tile as tile
    F32 = mybir.dt.float32
    BF16 = mybir.dt.bfloat16
    I16 = mybir.dt.int16
    AF = mybir.ActivationFunctionType
    OP = mybir.AluOpType
    AX = mybir.AxisListType

    B, NG, TFP, NGF, OUTN = (meta['B'], meta['NG'], meta['TFP'],
                             meta['NGF'], meta['OUTN'])
    FC2B = meta['fc2_b']
    HAS_BIAS = meta['has_bias']

    nc = bacc.Bacc(None, num_devices=NCORES, target_bir_lowering=False)
    P = lambda name, shape, dt: nc.declare_dram_parameter(name, shape, dt, isOutput=False)

    x_slice = P("x_slice", [SL + 128, 128], BF16)
    w1t = P("w1t", [128, 512], BF16)
    w2t = P("w2t", [128, 512], BF16)
    w3t = P("w3t", [128, 8], BF16)
    wsd1 = P("wsd1", [128, 8], BF16)
    wsd2 = P("wsd2", [128, 8], BF16)
    wsd3 = P("wsd3", [128, 2], BF16)
    ab = P("ab", [8, 32], F32)
    cb = P("cb", [128, 64], F32)
    w2b = P("w2b", [128, 16], F32)
    b1_bc = P("b1_bc", [128, 512], F32)
    b2_bc = P("b2_bc", [128, 128], F32)
    b3_bc = P("b3_bc", [128, 8], F32)
    iota_bf_in = P("iota_bf", [128, 128], BF16)
    ident_bf_in = P("ident_bf", [128, 128], BF16)
    ident_f_in = P("ident_f", [128, 128], F32)
    lo_idx = P("lo_idx", [NG, 128, G * TLO * 8], I16)
    hi_idx = P("hi_idx", [NG, 128, G * THI * 8], I16)
    dstloc = P("dstloc", [128, B * TT], F32)
    blkrow = P("blkrow", [NG, 128, G * 8], I16)
    fsrc = P("fsrc", [NGF, 128, GF * 8], I16)
    fdst = P("fdst", [NGF, 128, GF * 8], I16)
    msrc = P("msrc", [128, TFP], mybir.dt.int8)
    mdst = P("mdst", [128, TFP], mybir.dt.int8)
    attr = P("attr", [128, TFP * 4], F32)

    out_final = nc.declare_dram_parameter("out_final", [OUTN], F32, isOutput=True)

    x_bounce = nc.dram_tensor("x_bounce", [SL, 128], BF16)
    h2_slice = nc.dram_tensor("h2_slice", [SL + 128, 128], BF16)
    h3_slice = nc.dram_tensor("h3_slice", [SL + 128, 128], BF16)
    h3o_slice = nc.dram_tensor("h3o_slice", [SL + 128, 64], F32)
    h3_comp = nc.dram_tensor("h3_comp", [SL, 8], BF16)
    h3o_comp = nc.dram_tensor("h3o_comp", [SL, 8], F32)
    t_x = nc.dram_tensor("t_x", [NP, 128], BF16, addr_space="Shared")
    t_h2 = nc.dram_tensor("t_h2", [NP, 128], BF16, addr_space="Shared")
    h3_ag = nc.dram_tensor("h3_ag", [NP, 8], BF16, addr_space="Shared")
    h3o_ag = nc.dram_tensor("h3o_ag", [NP, 8], F32, addr_space="Shared")
    t_h3 = nc.dram_tensor("t_h3", [NP, 128], BF16)
    t_u = nc.dram_tensor("t_u", [NP // 2, 64], F32)
    t_v = nc.dram_tensor("t_v", [NP // 2, 64], F32)

    with tile.TileContext(nc) as tc:
        with tc.tile_pool(name="const", bufs=1) as cp, \
             tc.tile_pool(name="stage", bufs=2) as sp, \
             tc.tile_pool(name="work", bufs=2) as wp, \
             tc.tile_pool(name="psA", bufs=2, space="PSUM") as psA, \
             tc.tile_pool(name="psB", bufs=2, space="PSUM") as psB, \
             tc.tile_pool(name="psAcc", bufs=1, space="PSUM") as psAcc:

            _cn = [0]
            def load_const(ap, shape, dt):
                _cn[0] += 1
                t = cp.tile(shape, dt, tag=f"const{_cn[0]}")
                nc.sync.dma_start(out=t[:], in_=ap)
                return t

            c_w1t = load_const(w1t[:, :], [128, 512], BF16)
            c_w2t = load_const(w2t[:, :], [128, 512], BF16)
            c_w3t = load_const(w3t[:, :], [128, 8], BF16)
            c_wsd1 = load_const(wsd1[:, :], [128, 8], BF16)
            c_wsd2 = load_const(wsd2[:, :], [128, 8], BF16)
            c_wsd3 = load_const(wsd3[:, :], [128, 2], BF16)
            c_ab = load_const(ab[:, :], [8, 32], F32)
            c_cb = load_const(cb[:, :], [128, 64], F32)
            c_w2b = load_const(w2b[:, :], [128, 16], F32)
            c_b1 = load_const(b1_bc[:, :], [128, 512], F32)
            c_b2 = load_const(b2_bc[:, :], [128, 128], F32)
            c_b3 = load_const(b3_bc[:, :], [128, 8], F32)
            c_iota = load_const(iota_bf_in[:, :], [128, 128], BF16)
            c_idbf = load_const(ident_bf_in[:, :], [128, 128], BF16)
            c_idf = load_const(ident_f_in[:, :], [128, 128], F32)

            # zero scatter-target slices
            zt = cp.tile([128, 128], F32)
            nc.vector.memset(zt[:], 0.0)
            for tbl, wcols in ((h2_slice, 64), (h3_slice, 64), (h3o_slice, 64)):
                view = tbl[:, :].bitcast(F32)
                rows = SL + 128
                for r0 in range(0, rows, 128):
                    r1 = min(r0 + 128, rows)
                    nc.sync.dma_start(out=view[r0:r1, :], in_=zt[:r1 - r0, :wcols])

            def ts_copy(dst_ap, src_ap, tag=None, shape=None, dt=None):
                """PSUM->SBUF copy via tensor_scalar mult-by-1 (2x for bf16)."""
                nc.vector.tensor_scalar(out=dst_ap, in0=src_ap, scalar1=1.0,
                                        scalar2=None, op0=OP.mult)

            def elu_act(m_ps, bias_ap, width, out_dt):
                """ELU from PSUM pre-activation: 2 ACT + 1 DVE max + 1 DVE stt.
                m may have bias pre-added (HAS_BIAS path adds first)."""
                if HAS_BIAS:
                    mb = wp.tile([128, width], F32, tag=f"elmb{width}")
                    nc.vector.tensor_add(out=mb[:], in0=m_ps, in1=bias_ap)
                    m_ap = mb[:]
                else:
                    m_ap = m_ps
                r1 = wp.tile([128, width], F32, tag=f"elr1{width}")
                nc.scalar.activation(r1[:], m_ap, AF.Relu, scale=-1.0)
                e = wp.tile([128, width], F32, tag=f"ele{width}")
                nc.scalar.activation(e[:], r1[:], AF.Exp, scale=-1.0)
                r = wp.tile([128, width], F32, tag=f"elr{width}")
                nc.vector.tensor_scalar(out=r[:], in0=m_ap, scalar1=0.0,
                                        scalar2=None, op0=OP.max)
                out = wp.tile([128, width], out_dt, tag=f"elo{width}")
                nc.vector.scalar_tensor_tensor(
                    out=out[:], in0=e[:], scalar=-1.0, in1=r[:],
                    op0=OP.add, op1=OP.add)
                return out

            def edge_layer(layer, src_tbl, blk_tbl, scat_tbl, scat_elem):
                """Emit one GAT layer. Returns nothing; writes scat_tbl."""
                nH = 4 if layer != 3 else 1
                c_wsd = (c_wsd1, c_wsd2, c_wsd3)[layer - 1]
                for g in range(NG):
                    li = sp.tile([128, G * TLO * 8], I16, tag="li")
                    nc.sync.dma_start(out=li[:], in_=lo_idx[g])
                    hi = sp.tile([128, G * THI * 8], I16, tag="hi")
                    nc.sync.dma_start(out=hi[:], in_=hi_idx[g])
                    bi = sp.tile([128, G * 8], I16, tag="bi")
                    nc.sync.dma_start(out=bi[:], in_=blkrow[g])
                    dl = sp.tile([128, G * TT], F32, tag="dl")
                    nc.sync.dma_start(out=dl[:], in_=dstloc[:, g * G * TT:(g + 1) * G * TT])
                    stag_lo = sp.tile([128, G * TLO * 128], BF16, tag="stag_lo")
                    stag_hi = sp.tile([128, G * THI * 128], BF16, tag="stag_hi")
                    for bg in range(G):
                        nc.gpsimd.dma_gather(
                            out_ap=stag_lo[:, bg * TLO * 128:(bg + 1) * TLO * 128]
                                .rearrange("p (a b) -> p a b", a=TLO),
                            in_ap=src_tbl[0:HALF, :],
                            idxs_ap=li[:, bg * TLO * 8:(bg + 1) * TLO * 8],
                            num_idxs=TLO * 128, num_idxs_reg=TLO * 128,
                            elem_size=128)
                        nc.gpsimd.dma_gather(
                            out_ap=stag_hi[:, bg * THI * 128:(bg + 1) * THI * 128]
                                .rearrange("p (a b) -> p a b", a=THI),
                            in_ap=src_tbl[HALF:NP, :],
                            idxs_ap=hi[:, bg * THI * 8:(bg + 1) * THI * 8],
                            num_idxs=THI * 128, num_idxs_reg=THI * 128,
                            elem_size=128)
                    brow = sp.tile([128, G * 128], BF16, tag="brow")
                    nc.gpsimd.dma_gather(
                        out_ap=brow[:].rearrange("p (a b) -> p a b", a=G),
                        in_ap=blk_tbl[:, :], idxs_ap=bi[:],
                        num_idxs=G * 128, num_idxs_reg=G * 128, elem_size=128)

                    owid = 128 if layer != 3 else 64
                    ostage = sp.tile([128, G * owid],
                                     BF16 if layer != 3 else F32, tag="ost")

                    for bg in range(G):
                        xb = brow[:, bg * 128:(bg + 1) * 128]
                        # --- d per block node via PE: transpose + matmul ---
                        xbt_ps = psB.tile([128, 128], BF16, tag="b")
                        nc.tensor.transpose(out=xbt_ps[:], in_=xb, identity=c_idbf[:])
                        xbt = wp.tile([128, 128], BF16, tag="xbt_sb")
                        ts_copy(xbt[:], xbt_ps[:])
                        dblk_ps = psA.tile([128, nH], F32, tag="a")
                        nc.tensor.matmul(dblk_ps[:], xbt[:], c_wsd[:, nH:2 * nH],
                                         start=True, stop=True)
                        dblk_bf = wp.tile([128, nH], BF16, tag="dblk_bf")
                        nc.vector.tensor_copy(out=dblk_bf[:], in_=dblk_ps[:])

                        # --- pass 0 over tiles: m0 cache, exc (bf16), den/acc ---
                        m0c = wp.tile([128, TT * 128], BF16, tag="m0c")
                        exc = wp.tile([128, TT * nH], BF16, tag="exc")
                        if layer == 1:
                            den_ps = psAcc.tile([128, nH], F32, tag="den")
                            acc_ps = psAcc.tile([128, 512], F32, tag="acc")
                        else:
                            num_w = 132 if layer == 2 else 9
                            acc_ps = psAcc.tile([128, num_w], F32, tag="acc")
                        for t in range(TT):
                            sl0 = (bg * TLO + t) * 128 if t < TLO else (bg * THI + (t - TLO)) * 128
                            xg = (stag_lo if t < TLO else stag_hi)[:, sl0:sl0 + 128]
                            dcol = dl[:, bg * TT + t:bg * TT + t + 1]
                            m0 = m0c[:, t * 128:(t + 1) * 128]
                            nc.vector.tensor_scalar(
                                out=m0, in0=c_iota[:], scalar1=dcol,
                                scalar2=None, op0=OP.is_equal)
                            m0t_ps = psB.tile([128, 128], BF16, tag="b")
                            nc.tensor.transpose(out=m0t_ps[:], in_=m0, identity=c_idbf[:])
                            m0t = wp.tile([128, 128], BF16, tag="m0t_sb")
                            ts_copy(m0t[:], m0t_ps[:])
                            xgt_ps = psB.tile([128, 128], BF16, tag="b")
                            nc.tensor.transpose(out=xgt_ps[:], in_=xg, identity=c_idbf[:])
                            xgt = wp.tile([128, 128], BF16, tag="xgt_sb")
                            ts_copy(xgt[:], xgt_ps[:])
                            raw_ps = psA.tile([128, nH], F32, tag="a")
                            nc.tensor.matmul(raw_ps[:], xgt[:], c_wsd[:, 0:nH],
                                             start=True, stop=False)
                            nc.tensor.matmul(raw_ps[:], m0t[:], dblk_bf[:],
                                             start=False, stop=True)
                            lr = wp.tile([128, nH], F32, tag="lr")
                            nc.scalar.activation(lr[:], raw_ps[:], AF.Prelu, alpha=NEG)
                            exd = exc[:, t * nH:(t + 1) * nH]
                            nc.scalar.activation(exd, lr[:], AF.Exp)
                            first, last = (t == 0), (t == TT - 1)
                            if layer == 1:
                                nc.tensor.matmul(den_ps[:], m0, exd, start=first, stop=last)
                                gw = wp.tile([128, 512], BF16, tag="gw")
                                nc.vector.tensor_tensor(
                                    out=gw[:].rearrange("p (h f) -> p h f", h=4),
                                    in0=xg.rearrange("p (o f) -> p o f", o=1).to_broadcast([128, 4, 128]),
                                    in1=exd.rearrange("p (h o) -> p h o", o=1).to_broadcast([128, 4, 128]),
                                    op=OP.mult)
                                nc.tensor.matmul(acc_ps[:], m0, gw[:], start=first, stop=last)
                            else:
                                gw = wp.tile([128, num_w], BF16, tag="gw")
                                if layer == 2:
                                    nc.vector.tensor_tensor(
                                        out=gw[:, 0:128].rearrange("p (h c) -> p h c", h=4),
                                        in0=xg.rearrange("p (h c) -> p h c", h=4),
                                        in1=exd.rearrange("p (h o) -> p h o", o=1).to_broadcast([128, 4, 32]),
                                        op=OP.mult)
                                    ts_copy(gw[:, 128:132], exd)
                                else:
                                    nc.vector.tensor_tensor(
                                        out=gw[:, 0:8], in0=xg[:, 0:8],
                                        in1=exd.to_broadcast([128, 8]), op=OP.mult)
                                    ts_copy(gw[:, 8:9], exd)
                                nc.tensor.matmul(acc_ps[:], m0, gw[:], start=first, stop=last)

                        if layer == 1:
                            den_ap = den_ps[:]
                        elif layer == 2:
                            den_ap = acc_ps[:, 128:132]
                        else:
                            den_ap = acc_ps[:, 8:9]
                        den_sb = wp.tile([128, nH], F32, tag="den_sb")
                        nc.vector.tensor_scalar(out=den_sb[:], in0=den_ap, scalar1=1e-30,
                                                 scalar2=None, op0=OP.max)
                        rec = wp.tile([128, nH], F32, tag="rec")
                        nc.vector.reciprocal(out=rec[:], in_=den_sb[:])

                        if layer == 1:
                            # normalize + transform: nh = (acc/den) @ w1t per head
                            nh_ps = psA.tile([128, 512], F32, tag="nh")
                            pc = wp.tile([128, 512], BF16, tag="pc")
                            nc.vector.tensor_tensor(
                                out=pc[:].rearrange("p (h f) -> p h f", h=4),
                                in0=acc_ps[:].rearrange("p (h f) -> p h f", h=4),
                                in1=rec[:].rearrange("p (h o) -> p h o", o=1).to_broadcast([128, 4, 128]),
                                op=OP.mult)
                            for h in range(4):
                                pt_ps = psB.tile([128, 128], BF16, tag="b")
                                nc.tensor.transpose(out=pt_ps[:], in_=pc[:, h * 128:(h + 1) * 128],
                                                    identity=c_idbf[:])
                                pt = wp.tile([128, 128], BF16, tag="pt_sb")
                                ts_copy(pt[:], pt_ps[:])
                                nc.tensor.matmul(nh_ps[:, h * 128:(h + 1) * 128],
                                                 pt[:], c_w1t[:, h * 128:(h + 1) * 128],
                                                 start=True, stop=True)
                            elu1 = elu_act(nh_ps[:], c_b1[:], 512, BF16)
                            # dense2 -> h2 block
                            h2_ps = psA.tile([128, 128], F32, tag="a")
                            for c in range(4):
                                ct_ps = psB.tile([128, 128], BF16, tag="b")
                                nc.tensor.transpose(out=ct_ps[:], in_=elu1[:, c * 128:(c + 1) * 128],
                                                    identity=c_idbf[:])
                                ct = wp.tile([128, 128], BF16, tag="ct_sb")
                                ts_copy(ct[:], ct_ps[:], "ct", None, None)
                                nc.tensor.matmul(h2_ps[:], ct[:], c_w2t[:, c * 128:(c + 1) * 128],
                                                 start=(c == 0), stop=(c == 3))
                            nc.vector.tensor_copy(out=ostage[:, bg * 128:(bg + 1) * 128],
                                                  in_=h2_ps[:])
                        elif layer == 2:
                            o_sb = wp.tile([128, 128], F32, tag="o_sb2")
                            nc.vector.tensor_tensor(
                                out=o_sb[:].rearrange("p (h c) -> p h c", h=4),
                                in0=acc_ps[:, 0:128].rearrange("p (h c) -> p h c", h=4),
                                in1=rec[:].rearrange("p (h o) -> p h o", o=1).to_broadcast([128, 4, 32]),
                                op=OP.mult)
                            elu2 = elu_act(o_sb[:], c_b2[:], 128, BF16)
                            ct_ps = psB.tile([128, 128], BF16, tag="b")
                            nc.tensor.transpose(out=ct_ps[:], in_=elu2[:], identity=c_idbf[:])
                            ct = wp.tile([128, 128], BF16, tag="ct_sb")
                            ts_copy(ct[:], ct_ps[:], "ct2", None, None)
                            h3_ps = psA.tile([128, 8], F32, tag="a")
                            nc.tensor.matmul(h3_ps[:], ct[:], c_w3t[:], start=True, stop=True)
                            st = ostage[:, bg * 128:(bg + 1) * 128]
                            nc.vector.memset(st, 0.0)
                            nc.vector.tensor_copy(out=ostage[:, bg * 128:bg * 128 + 8],
                                                  in_=h3_ps[:])
                        else:
                            o_sb = wp.tile([128, 8], F32, tag="o_sb3")
                            nc.vector.tensor_tensor(
                                out=o_sb[:], in0=acc_ps[:, 0:8],
                                in1=rec[:].to_broadcast([128, 8]), op=OP.mult)
                            elu3 = elu_act(o_sb[:], c_b3[:], 8, F32)
                            st = ostage[:, bg * 64:(bg + 1) * 64]
                            nc.vector.memset(st, 0.0)
                            nc.vector.tensor_copy(out=ostage[:, bg * 64:bg * 64 + 8],
                                                  in_=elu3[:])

                    nc.gpsimd.dma_scatter_add(
                        scat_tbl[:, :], ostage[:].rearrange("p (a b) -> p a b", a=G),
                        bi[:], G * 128, G * 128, scat_elem)

            # ======== layers ========
            nc.sync.dma_start(out=x_bounce[:, :], in_=x_slice[0:SL, :])
            nc.gpsimd.collective_compute(
                "AllGather", mybir.AluOpType.bypass,
                replica_groups=[list(range(NCORES))],
                ins=[x_bounce[:, :]], outs=[t_x[:, :]])
            edge_layer(1, t_x, x_slice, h2_slice, 128)
            nc.gpsimd.collective_compute(
                "AllGather", mybir.AluOpType.bypass,
                replica_groups=[list(range(NCORES))],
                ins=[h2_slice[0:SL, :]], outs=[t_h2[:, :]])

            edge_layer(2, t_h2, h2_slice, h3_slice, 128)
            for r0 in range(0, SL, 512):
                r1 = min(r0 + 512, SL)
                nc.sync.dma_start(out=h3_comp[r0:r1, :],
                                  in_=h3_slice[r0:r1, 0:8])
            nc.gpsimd.collective_compute(
                "AllGather", mybir.AluOpType.bypass,
                replica_groups=[list(range(NCORES))],
                ins=[h3_comp[:, :]], outs=[h3_ag[:, :]])
            for r0 in range(0, NP, 512):
                r1 = min(r0 + 512, NP)
                nc.sync.dma_start(out=t_h3[r0:r1, 0:8],
                                  in_=h3_ag[r0:r1, :])

            edge_layer(3, t_h3, h3_slice, h3o_slice, 64)
            for r0 in range(0, SL, 512):
                r1 = min(r0 + 512, SL)
                nc.sync.dma_start(out=h3o_comp[r0:r1, :],
                                  in_=h3o_slice[r0:r1, 0:8])
            nc.gpsimd.collective_compute(
                "AllGather", mybir.AluOpType.bypass,
                replica_groups=[list(range(NCORES))],
                ins=[h3o_comp[:, :]], outs=[h3o_ag[:, :]])

            # ======== u/v tables (pack-2 rows) ========
            for r in range(NP // 128):
                hrows = wp.tile([128, 8], F32, tag="hrows")
                nc.sync.dma_start(out=hrows[:], in_=h3o_ag[r * 128:(r + 1) * 128, :])
                ht_ps = psA.tile([128, 128], F32, tag="a")
                nc.tensor.transpose(out=ht_ps[0:8, :], in_=hrows[:], identity=c_idf[:])
                ht = wp.tile([8, 128], F32, tag="ht_sb")
                nc.vector.tensor_copy(out=ht[:], in_=ht_ps[0:8, :])
                uv_ps = psA.tile([128, 32], F32, tag="a")
                nc.tensor.matmul(uv_ps[:], ht[:], c_ab[:], start=True, stop=True)
                ust = wp.tile([128, 16], F32, tag="ust")
                nc.vector.tensor_copy(out=ust[:], in_=uv_ps[:, 0:16])
                vst = wp.tile([128, 16], F32, tag="vst")
                nc.vector.tensor_copy(out=vst[:], in_=uv_ps[:, 16:32])
                du = t_u[:, :].rearrange("r (s c) -> r s c", s=2)
                nc.sync.dma_start(out=du[r * 64:(r + 1) * 64, :, 0:16], in_=ust[:])
                dv = t_v[:, :].rearrange("r (s c) -> r s c", s=2)
                nc.sync.dma_start(out=dv[r * 64:(r + 1) * 64, :, 0:16], in_=vst[:])

            # ======== final edge MLP ========
            fout = sp.tile([128, 128], F32, tag="fout")
            nsup = 0
            for g in range(NGF):
                fi = sp.tile([128, GF * 8], I16, tag="fi")
                nc.sync.dma_start(out=fi[:], in_=fsrc[g])
                di = sp.tile([128, GF * 8], I16, tag="di")
                nc.sync.dma_start(out=di[:], in_=fdst[g])
                ug = sp.tile([128, GF * 64], F32, tag="ug")
                nc.gpsimd.dma_gather(
                    out_ap=ug[:].rearrange("p (a b) -> p a b", a=GF),
                    in_ap=t_u[:, :], idxs_ap=fi[:],
                    num_idxs=GF * 128, num_idxs_reg=GF * 128, elem_size=64)
                vg = sp.tile([128, GF * 64], F32, tag="vg")
                nc.gpsimd.dma_gather(
                    out_ap=vg[:].rearrange("p (a b) -> p a b", a=GF),
                    in_ap=t_v[:, :], idxs_ap=di[:],
                    num_idxs=GF * 128, num_idxs_reg=GF * 128, elem_size=64)
                ms = sp.tile([128, GF], mybir.dt.int8, tag="ms")
                nc.sync.dma_start(out=ms[:], in_=msrc[:, g * GF:(g + 1) * GF])
                md = sp.tile([128, GF], mybir.dt.int8, tag="md")
                nc.sync.dma_start(out=md[:], in_=mdst[:, g * GF:(g + 1) * GF])
                at = sp.tile([128, GF * 4], F32, tag="at")
                nc.sync.dma_start(out=at[:], in_=attr[:, g * GF * 4:(g + 1) * GF * 4])

                for t in range(GF):
                    gt = g * GF + t
                    usel = wp.tile([128, 16], F32, tag="usel")
                    nc.vector.select(out=usel[:], mask=ms[:, t:t + 1].to_broadcast([128, 16]),
                                     on_true=ug[:, t * 64 + 32:t * 64 + 48],
                                     on_false=ug[:, t * 64:t * 64 + 16])
                    vsel = wp.tile([128, 16], F32, tag="vsel")
                    nc.vector.select(out=vsel[:], mask=md[:, t:t + 1].to_broadcast([128, 16]),
                                     on_true=vg[:, t * 64 + 32:t * 64 + 48],
                                     on_false=vg[:, t * 64:t * 64 + 16])
                    m1 = wp.tile([128, 16], F32, tag="m1")
                    nc.vector.tensor_add(out=m1[:], in0=usel[:], in1=vsel[:])
                    for j in range(4):
                        nc.vector.scalar_tensor_tensor(
                            out=m1[:], in0=c_cb[:, j * 16:(j + 1) * 16],
                            scalar=at[:, t * 4 + j:t * 4 + j + 1],
                            in1=m1[:], op0=OP.mult, op1=OP.add)
                    rr = wp.tile([128, 16], F32, tag="rr")
                    nc.scalar.activation(rr[:], m1[:], AF.Relu)
                    rw = wp.tile([128, 16], F32, tag="rw")
                    nc.vector.tensor_tensor(out=rw[:], in0=rr[:], in1=c_w2b[:], op=OP.mult)
                    nc.vector.tensor_reduce(
                        out=fout[:, gt % 128:gt % 128 + 1], in_=rw[:], axis=AX.X, op=OP.add)

                if (g + 1) % (128 // GF) == 0 or g == NGF - 1:
                    fo2 = wp.tile([128, 128], F32, tag="fo2")
                    nc.vector.tensor_scalar(out=fo2[:], in0=fout[:], scalar1=FC2B,
                                            scalar2=None, op0=OP.add)
                    ft_ps = psA.tile([128, 128], F32, tag="a")
                    nc.tensor.transpose(out=ft_ps[:], in_=fo2[:], identity=c_idf[:])
                    ft = wp.tile([128, 128], F32, tag="ft_sb")
                    nc.vector.tensor_copy(out=ft[:], in_=ft_ps[:])
                    o0 = nsup * 16384
                    nc.sync.dma_start(
                        out=out_final[o0:o0 + 16384].rearrange("(a b) -> a b", b=128),
                        in_=ft[:])
                    nsup += 1
                    if g != NGF - 1:
                        fout = sp.tile([128, 128], F32, tag="fout")

    return nc


def kernel(**inputs):
    inputs = {k: np.asarray(v) for k, v in inputs.items()}
    in_maps, meta = _host_prep(**inputs)
    nc = _build(meta)
    nc.finalize()
    from concourse.bass_utils import run_bass_kernel_spmd
    res = run_bass_kernel_spmd(nc, in_maps, list(range(NCORES))).results
    EC = meta['EC']
    out = np.concatenate([res[k]["out_final"][:EC] for k in range(NCORES)])
    return out[:E].astype(np.float32)
